# revision 30
# baseline (speedup 1.0000x reference)
"""GAT (2-layer, PyG-style) distributed Bass kernel for 8 TRN2 NeuronCores.

Strategy (sharding_hint: 1D node partition by dst), v3:
  - core c owns dst nodes [c*NPC, (c+1)*NPC).
  - dense phase: each core computes table1 rows [h1(64)|a_src1(8)|pad->256B]
    for its node slice; AllGather -> full padded table in every core's DRAM.
    A position-ordered copy [h|asrc|adst] (ad1pos) is written in parallel by
    dma_scatter_add (1024 int16 indices/call; rows unique so add==write on
    the zeroed table).
  - edge phase: self-loop edges are NOT materialized (handled analytically in
    the post-pass, position-aligned).  Remaining edges: host packs dst nodes
    into BLOCKS of <=128 nodes whose edges, split by SOURCE SHARD, fit 2
    single-shard chunks of 128 slots per shard (8-dim FFD; balanced because
    self-loops no longer skew the local shard).  4 blocks = 1 supertile = 64
    chunks, chunk c = shard*8 + block_loc*2 + half.  Per supertile:
      * 8 dma_gather calls (Q7 custom op, 1024 int16 shard-local indices,
        256B rows, spread over 4 SWDGE queues -> ~0.9us/call) pull all 8192
        edge source rows.  vs ~1us per 128-row indirect DMA = ~9x fewer
        Pool-engine descriptor-generation stalls.
      * per-edge a_dst: eadst[e,h] = sposT_chunk^T @ ad_block on TensorE,
        where sposT (pos->edge one-hot) is built by iota/is_equal from a
        K=1 ones-matmul broadcast of the dst-position vector dloc.
      * w = exp(leakyrelu(a_src+a_dst)) (no max subtraction; |logit| < ~3),
        hw = [h*w | w], then per block 16 accumulating matmuls
        blk[128pos, nw] += spos_chunk^T @ hw_chunk (PSUM).
  - block post: fold in the self-loop term (w_sl = exp(lrelu(asrc+adst)),
    num += w_sl*h, den += w_sl) from the position-ordered table, normalize,
    +bias, ELU, W2aug matmul -> layer-2 node rows (dma_scatter_add into the
    zeroed padded table) + position-ordered copy; AllGather #2; same edge
    pass for layer 2; log_softmax; output rows in position order (host
    unpacks by posmap).
All floating-point math runs on-device.  Host work is integer graph
preprocessing (sort/pack/index building) and weight layout rearrangement.
"""
import os
import sys
import numpy as np

try:
    import concourse.bass as bass
except ImportError:  # pragma: no cover
    for p in ("/opt/trn_rl_repo", "/root/.axon_site/_ro/trn_rl_repo"):
        if os.path.isdir(p) and p not in sys.path:
            sys.path.insert(0, p)
    import concourse.bass as bass

import ml_dtypes
import concourse.mybir as mybir
import concourse.tile as tile
import concourse.bacc as bacc
from concourse import library_config
from concourse.masks import make_identity

BF16 = ml_dtypes.bfloat16
DT = mybir.dt

# ---------------- problem config (hardcoded per contract) ----------------
N, E, F = 100000, 1600000, 256
H1, C1 = 8, 8          # layer1 heads x channels (concat -> 64)
C2 = 40                # layer2 single head, 40 classes
NEG = 0.2
NCORES = 8
NPC = N // NCORES      # 12500
NPCT = 12672           # table rows per shard (12544 P1-padded + 128 dump)
ROWP = 128             # padded table row, bf16 elements (= 256B)
TILE_E = 128           # edge slots per tile/chunk
SPB = 2                # chunks per (block, shard)
BPS = 4                # blocks per supertile
K = 64                 # chunks per supertile = 8 shards x 4 blocks x 2
ROW1 = 72              # useful cols layer1: [h1(64) | asrc1(8)]
ROW2 = 41              # useful cols layer2: [h2(40) | asrc2(1)]
NP1T = 99              # P1 tiles (12672 rows / 128)
NP1G = 13              # P1 scatter groups of 8 tiles

_f32 = np.float32


# =================== host-side graph preprocessing ===================

def _pack_blocks(deg2):
    """8-dim FFD: pack nodes into blocks (<=128 nodes, per-shard edge count
    <=SPB*TILE_E).  deg2: [NPC, 8] per-node per-source-shard edge counts."""
    cap = SPB * TILE_E
    order = np.argsort(-deg2.sum(1), kind="stable")
    blocks = []
    rem = np.zeros((0, 8), np.int64)
    npos = np.zeros(0, np.int64)
    open_ids = np.zeros(0, np.int64)
    for n in order:
        dn = deg2[n]
        ok = np.nonzero((rem >= dn).all(1) & (npos < 128))[0]
        if ok.size:
            k = ok[0]
            bi = open_ids[k]
            blocks[bi].append(int(n))
            rem[k] -= dn
            npos[k] += 1
            if npos[k] >= 128 or rem[k].sum() < 3:
                keep = np.arange(rem.shape[0]) != k
                rem, npos, open_ids = rem[keep], npos[keep], open_ids[keep]
        else:
            blocks.append([int(n)])
            rem = np.vstack([rem, (cap - dn)[None]])
            npos = np.append(npos, 1)
            open_ids = np.append(open_ids, len(blocks) - 1)
        if rem.shape[0] > 64:
            drop = np.argmin(rem.sum(1))
            keep = np.arange(rem.shape[0]) != drop
            rem, npos, open_ids = rem[keep], npos[keep], open_ids[keep]
    return blocks


def _wrap16(vals):
    """[n] -> wrapped int16 [128, n/16] layout: index i at [i%16, i//16],
    replicated across the 8 groups of 16 partitions."""
    n = vals.shape[-1]
    v = vals.reshape(*vals.shape[:-1], n // 16, 16)
    v = np.moveaxis(v, -1, -2)  # [..., 16, n//16]
    v = np.concatenate([v] * 8, axis=-2)  # tile to 128 partitions
    return np.ascontiguousarray(v).astype(np.int16)


def preprocess(edge_index):
    # self-loops are handled analytically in the post-pass (position-aligned)
    src = edge_index[0].astype(np.int64)
    dst = edge_index[1].astype(np.int64)

    cores = []
    max_nb = 0
    for c in range(NCORES):
        lo = c * NPC
        m = (dst >= lo) & (dst < lo + NPC)
        s_c, d_c = src[m], dst[m] - lo
        shard = s_c // NPC
        srow = s_c % NPC
        key = d_c * 8 + shard
        order = np.argsort(key, kind="stable")
        srow_s = srow[order]
        deg2 = np.bincount(key, minlength=NPC * 8).reshape(NPC, 8)
        starts = np.zeros(NPC * 8 + 1, np.int64)
        starts[1:] = np.cumsum(deg2.reshape(-1))
        blocks = _pack_blocks(deg2)
        cores.append(dict(srow_s=srow_s, starts=starts, blocks=blocks))
        max_nb = max(max_nb, len(blocks))

    nblocks = -(-max_nb // BPS) * BPS
    assert nblocks * 128 + 128 <= 32767, f"NPOS overflow: {nblocks}"
    S = nblocks // BPS
    NPOS = nblocks * 128
    DUMP = 12544       # dump row base for t2loc (rows [12544, 12672))

    per_core = []
    for cd in cores:
        blocks, starts, srow_s = cd["blocks"], cd["starts"], cd["srow_s"]
        blocks = blocks + [[] for _ in range(nblocks - len(blocks))]
        posmap = np.full((nblocks, 128), -1, np.int64)
        # node/pad -> position; pads & tail -> spread dump positions
        pscat = NPOS + (np.arange(NP1G * 8 * 128, dtype=np.int64) % 128)
        idx_loc = np.zeros((nblocks, 8, SPB * TILE_E), np.int64)
        dpos = np.full((nblocks, 8, SPB * TILE_E), 255, np.int64)
        for b, nodes in enumerate(blocks):
            ptr = np.zeros(8, np.int64)
            for pos, n in enumerate(nodes):
                posmap[b, pos] = n
                pscat[n] = b * 128 + pos
                for sg in range(8):
                    a, z = starts[n * 8 + sg], starts[n * 8 + sg + 1]
                    k = z - a
                    if k:
                        p0 = ptr[sg]
                        idx_loc[b, sg, p0:p0 + k] = srow_s[a:z]
                        dpos[b, sg, p0:p0 + k] = pos
                        ptr[sg] += k
            assert (ptr <= SPB * TILE_E).all()
        # chunk layout: global chunk c = sg*8 + b_loc*SPB + half; call sg
        # covers chunks [sg*8, sg*8+8) = its supertile's 4 blocks x 2 halves
        idx4 = idx_loc.reshape(S, BPS, 8, SPB * TILE_E).transpose(0, 2, 1, 3)
        idxw = _wrap16(idx4.reshape(S, 8, 8 * TILE_E)).reshape(S, 8, 128, 64)
        idxw = np.ascontiguousarray(idxw.transpose(0, 2, 1, 3)).reshape(
            S, 128, 512)
        # dl [S, 128slot, 64chunk]
        dlc = dpos.reshape(S, BPS, 8, SPB, TILE_E).transpose(0, 2, 1, 3, 4)
        dl = np.ascontiguousarray(
            dlc.reshape(S, K, TILE_E).transpose(0, 2, 1)).astype(BF16)
        # dlT [S, 1, 64*128]: [0, c*128 + slot] = dloc(chunk c, slot)
        dlT = np.ascontiguousarray(
            dlc.reshape(S, 1, K * TILE_E)).astype(BF16)
        # rowiw [S, 128, 32]: scatter idx for post blocks: i=b_loc*128+pos
        rowv = posmap.reshape(S, BPS * 128).copy()
        bad = rowv < 0
        rowv[bad] = DUMP + (np.nonzero(bad)[1] % 128)
        rowiw = _wrap16(rowv)
        # pscatw [NP1G, 128, 64]: P1 tile j covers nodes j*128..j*128+127
        pv = pscat[:NP1G * 8 * 128].reshape(NP1G, 8 * 128)
        pscatw = _wrap16(pv)
        per_core.append(dict(idxw=idxw, dl=dl, dlT=dlT, rowiw=rowiw,
                             pscatw=pscatw,
                             posmap_flat=posmap.reshape(-1).copy()))
    meta = dict(S=S, nblocks=nblocks, NPOS=NPOS)
    return meta, per_core


def build_weight_inputs(W1, att_src1, att_dst1, bias1, W2, att_src2, att_dst2,
                        bias2):
    """Pure layout rearrangement of weights (no FP arithmetic)."""
    A1 = np.zeros((64, 16), _f32)
    for h in range(H1):
        A1[h * 8:(h + 1) * 8, h] = att_src1[h]
        A1[h * 8:(h + 1) * 8, 8 + h] = att_dst1[h]
    att2 = np.concatenate([att_src2.T, att_dst2.T], axis=1).astype(_f32)
    b1r = np.broadcast_to(bias1.astype(_f32), (128, 64)).copy()
    b2r = np.broadcast_to(bias2.astype(_f32), (128, C2)).copy()
    return dict(W1=W1.astype(_f32), A1=A1, W2=W2.astype(_f32), att2=att2,
                b1r=b1r, b2r=b2r)


# =================== device program ===================

def _brd(ap, pattern, off=0):
    """Manual AP: keep partition dim, explicit free-dim [step,count] pattern."""
    return bass.AP(ap.tensor, ap.offset + off, [ap.ap[0]] + pattern)


def _ap3(t, chunks, elem, off=0):
    a = t[:]
    return bass.AP(a.tensor, a.offset + off, [a.ap[0], [elem, chunks],
                                              [1, elem]])


def build_program(meta):
    S, NPOS = meta["S"], meta["NPOS"]
    NT = NPCT * NCORES

    nc = bacc.Bacc("TRN2", target_bir_lowering=False, debug=False,
                   enable_asserts=False, num_devices=NCORES,
                   num_swdge_queues=4)

    def din(name, shape, dt):
        return nc.dram_tensor(name, shape, dt, kind="ExternalInput").ap()

    x_sl = din("x_sl", [NP1T * 128, F], DT.bfloat16)
    W1 = din("W1", [F, 64], DT.float32)
    A1 = din("A1", [64, 16], DT.float32)
    W2 = din("W2", [64, C2], DT.float32)
    att2 = din("att2", [C2, 2], DT.float32)
    b1r = din("b1r", [128, 64], DT.float32)
    b2r = din("b2r", [128, C2], DT.float32)
    idxw_d = din("idxw", [S, 128, 512], DT.int16)
    dl_d = din("dl", [S, 128, K], DT.bfloat16)
    dlT_d = din("dlT", [S, 1, K * 128], DT.bfloat16)
    rowiw_d = din("rowiw", [S, 128, 32], DT.int16)
    pscatw_d = din("pscatw", [NP1G, 128, 64], DT.int16)

    out_d = nc.dram_tensor("out", [NPOS, C2], DT.float32,
                           kind="ExternalOutput").ap()

    t1loc = nc.dram_tensor("t1loc", [NPCT, ROWP], DT.bfloat16).ap()
    t1full = nc.dram_tensor("t1full", [NT, ROWP], DT.bfloat16,
                            addr_space="Shared").ap()
    t2loc = nc.dram_tensor("t2loc", [NPCT, ROWP], DT.bfloat16).ap()
    t2full = nc.dram_tensor("t2full", [NT, ROWP], DT.bfloat16,
                            addr_space="Shared").ap()
    ad1pos = nc.dram_tensor("ad1pos", [NPOS + 128, ROWP], DT.bfloat16).ap()
    ad2pos = nc.dram_tensor("ad2pos", [NPOS, 64], DT.bfloat16).ap()

    groups = [list(range(NCORES))]

    with tile.TileContext(nc, num_cores=NCORES) as tc:
        from contextlib import ExitStack
        with ExitStack() as top:
            cpool = top.enter_context(tc.tile_pool(name="const", bufs=1))
            id_f = cpool.tile([128, 128], DT.float32)
            make_identity(nc, id_f[:])
            id_b = cpool.tile([128, 128], DT.bfloat16)
            nc.vector.tensor_copy(id_b[:], id_f[:])
            ioF_i = cpool.tile([128, 128], DT.int16)
            nc.gpsimd.iota(ioF_i[:], pattern=[[1, 128]], base=0,
                           channel_multiplier=0)
            iotaF = cpool.tile([128, 128], DT.bfloat16)
            nc.vector.tensor_copy(iotaF[:], ioF_i[:])
            ioP_i = cpool.tile([128, 1], DT.int16)
            nc.gpsimd.iota(ioP_i[:], pattern=[[0, 1]], base=0,
                           channel_multiplier=1)
            iotaP = cpool.tile([128, 1], DT.bfloat16)
            nc.vector.tensor_copy(iotaP[:], ioP_i[:])
            ones1 = cpool.tile([1, 128], DT.bfloat16)
            nc.vector.memset(ones1[:], 1.0)
            b1sb = cpool.tile([128, 64], DT.float32)
            nc.sync.dma_start(b1sb[:], b1r)
            b2sb = cpool.tile([128, C2], DT.float32)
            nc.sync.dma_start(b2sb[:], b2r)
            # switch Q7 library: enables dma_gather / dma_scatter_add
            nc.gpsimd.load_library(library_config.mlp)

            # ---------- P0: weight prep ----------
            rhs1 = [cpool.tile([128, 80], DT.bfloat16, tag=f"rhs1_{i}",
                               name=f"rhs1_{i}") for i in range(2)]
            rhs2 = cpool.tile([64, 42], DT.bfloat16)
            with tc.tile_pool(name="p0", bufs=1) as p0, \
                 tc.tile_pool(name="p0ps", bufs=1, space="PSUM") as p0ps:
                w1sb = [p0.tile([128, 64], DT.float32, tag=f"w1_{i}",
                                name=f"w1_{i}") for i in range(2)]
                for i in range(2):
                    nc.sync.dma_start(w1sb[i][:], W1[128 * i:128 * (i + 1), :])
                a1sb = p0.tile([64, 16], DT.float32)
                nc.sync.dma_start(a1sb[:], A1)
                w2sb = p0.tile([64, C2], DT.float32)
                nc.sync.dma_start(w2sb[:], W2)
                at2sb = p0.tile([C2, 2], DT.float32)
                nc.sync.dma_start(at2sb[:], att2)
                for i in range(2):
                    tp = p0ps.tile([64, 128], DT.float32, tag="w1t_ps")
                    nc.tensor.transpose(tp[:], w1sb[i][:], id_f[:])
                    w1t = p0.tile([64, 128], DT.float32, tag="w1t")
                    nc.vector.tensor_copy(w1t[:], tp[:])
                    wa = p0ps.tile([128, 16], DT.float32, tag="w1a_ps")
                    nc.tensor.matmul(wa[:], lhsT=w1t[:], rhs=a1sb[:],
                                     start=True, stop=True)
                    nc.vector.tensor_copy(rhs1[i][:, 0:64], w1sb[i][:])
                    nc.vector.tensor_copy(rhs1[i][:, 64:80], wa[:])
                tp2 = p0ps.tile([C2, 64], DT.float32, tag="w2t_ps")
                nc.tensor.transpose(tp2[:], w2sb[:], id_f[:64, :64])
                w2t = p0.tile([C2, 64], DT.float32)
                nc.vector.tensor_copy(w2t[:], tp2[:])
                wa2 = p0ps.tile([64, 2], DT.float32, tag="w2a_ps")
                nc.tensor.matmul(wa2[:], lhsT=w2t[:], rhs=at2sb[:],
                                 start=True, stop=True)
                nc.vector.tensor_copy(rhs2[:, 0:C2], w2sb[:])
                nc.vector.tensor_copy(rhs2[:, C2:C2 + 2], wa2[:])

            # ---------- P0.5: zero scatter-add target tables ----------
            with tc.tile_pool(name="pz", bufs=1) as pz:
                ZW = 4096
                zt = pz.tile([128, ZW], DT.bfloat16)
                nc.vector.memset(zt[:], 0.0)
                for tgt, nelem in ((ad1pos, (NPOS + 128) * ROWP),
                                   (ad2pos, NPOS * 64),
                                   (t2loc, NPCT * ROWP)):
                    done = 0
                    while done < nelem:
                        chunk = min(ZW * 128, nelem - done)
                        w = chunk // 128
                        nc.sync.dma_start(
                            bass.AP(tgt.tensor, done, [[w, 128], [1, w]]),
                            zt[:, 0:w])
                        done += w * 128

            # ---------- P1: dense layer-1 table ----------
            with tc.tile_pool(name="p1", bufs=3) as p1, \
                 tc.tile_pool(name="p1ps", bufs=2, space="PSUM") as p1ps:
                sta8 = opw = None
                for it in range(NP1T):
                    g, gi = divmod(it, 8)
                    if gi == 0:
                        sta8 = p1.tile([128, 8 * ROWP], DT.bfloat16,
                                       tag="sta8", name="sta8")
                        nc.vector.memset(sta8[:], 0.0)
                        opw = p1.tile([128, 64], DT.int16, tag="opw")
                        nc.sync.dma_start(opw[:], pscatw_d[g])
                    xb = p1.tile([128, F], DT.bfloat16, tag="xb")
                    nc.sync.dma_start(xb[:], x_sl[128 * it:128 * (it + 1), :])
                    xT = p1.tile([128, F], DT.bfloat16, tag="xT")
                    ps1 = p1ps.tile([128, 80], DT.float32, tag="ps1")
                    for i in range(2):
                        tp = p1ps.tile([128, 128], DT.bfloat16, tag="xt_ps")
                        nc.tensor.transpose(
                            tp[:], xb[:, 128 * i:128 * (i + 1)], id_b[:])
                        nc.scalar.copy(xT[:, 128 * i:128 * (i + 1)], tp[:])
                    for i in range(2):
                        nc.tensor.matmul(
                            ps1[:], lhsT=xT[:, 128 * i:128 * (i + 1)],
                            rhs=rhs1[i][:], start=(i == 0), stop=(i == 1))
                    st = p1.tile([128, ROWP], DT.bfloat16, tag="st1")
                    nc.vector.memset(st[:, ROW1:ROWP], 0.0)
                    nc.scalar.copy(st[:, 0:ROW1], ps1[:, 0:ROW1])
                    nc.scalar.copy(sta8[:, ROWP * gi:ROWP * gi + 80],
                                   ps1[:, 0:80])
                    nc.sync.dma_start(t1loc[128 * it:128 * (it + 1), :], st[:])
                    if gi == 7 or it == NP1T - 1:
                        nc.gpsimd.dma_scatter_add(
                            ad1pos, _ap3(sta8, 8, ROWP), opw[:],
                            1024, 1024, ROWP, queue_num=g % 4)

            # ---------- P2: AllGather table1 ----------
            nc.gpsimd.collective_compute(
                "AllGather", mybir.AluOpType.bypass, replica_groups=groups,
                ins=[t1loc.opt()], outs=[t1full.opt()])

            # ---------- P3: edge pass layer 1 ----------
            edge_pass(nc, tc, meta, 1, idxw_d, dl_d, dlT_d, rowiw_d,
                      t1full, ad1pos, t2loc, ad2pos, None,
                      iotaF, iotaP, ones1, id_b, b1sb, rhs2)

            # ---------- P4: AllGather table2 ----------
            nc.gpsimd.collective_compute(
                "AllGather", mybir.AluOpType.bypass, replica_groups=groups,
                ins=[t2loc.opt()], outs=[t2full.opt()])

            # ---------- P5: edge pass layer 2 ----------
            edge_pass(nc, tc, meta, 2, idxw_d, dl_d, dlT_d, rowiw_d,
                      t2full, ad2pos, None, None, out_d,
                      iotaF, iotaP, ones1, id_b, b2sb, None)

    nc.compile()
    return nc


def edge_pass(nc, tc, meta, layer, idxw_d, dl_d, dlT_d, rowiw_d, tfull,
              adpos, t2loc, ad2pos, out_d, iotaF, iotaP, ones1, id_b,
              bias_sb, rhs2):
    S = meta["S"]
    nh = H1 if layer == 1 else 1          # heads
    nch = 64 if layer == 1 else C2        # message channels
    asrc_c = nch                          # a_src column in table row
    nw = nch + nh                         # hw width: [msgs*w | w]
    from contextlib import ExitStack
    with ExitStack() as ctx:
        pm = ctx.enter_context(tc.tile_pool(name=f"e{layer}m", bufs=2))
        pg = ctx.enter_context(tc.tile_pool(name=f"e{layer}g", bufs=2))
        pw = ctx.enter_context(tc.tile_pool(name=f"e{layer}w", bufs=2))
        pb = ctx.enter_context(tc.tile_pool(name=f"e{layer}b", bufs=2))
        psT = ctx.enter_context(
            tc.tile_pool(name=f"e{layer}pT", bufs=1, space="PSUM"))
        psE = ctx.enter_context(
            tc.tile_pool(name=f"e{layer}pE", bufs=2, space="PSUM"))
        psB = ctx.enter_context(
            tc.tile_pool(name=f"e{layer}pB", bufs=2, space="PSUM"))
        psP = ctx.enter_context(
            tc.tile_pool(name=f"e{layer}pP", bufs=1, space="PSUM"))
        state = {}
        for s in range(S):
            idx = pm.tile([128, 512], DT.int16, tag="idx")
            nc.sync.dma_start(idx[:], idxw_d[s])
            dl = pm.tile([128, K], DT.bfloat16, tag="dl")
            nc.sync.dma_start(dl[:], dl_d[s])
            dlT = pm.tile([1, K * 128], DT.bfloat16, tag="dlT")
            nc.sync.dma_start(dlT[:], dlT_d[s])
            rwi = pm.tile([128, 32], DT.int16, tag="rwi")
            if layer == 1:
                nc.sync.dma_start(rwi[:], rowiw_d[s])
            SLW = 80 if layer == 1 else 42
            RW2 = ROWP if layer == 1 else 64
            ad = pm.tile([128, BPS * nh], DT.bfloat16, tag="ad")
            adsl = pm.tile([128, BPS * SLW], DT.bfloat16, tag="adsl")
            adoff = 72 if layer == 1 else 41
            nc.sync.dma_start(
                ad[:], bass.AP(adpos.tensor, s * BPS * 128 * RW2 + adoff,
                               [[RW2, 128], [128 * RW2, BPS], [1, nh]]))
            nc.sync.dma_start(
                adsl[:], bass.AP(adpos.tensor, s * BPS * 128 * RW2,
                                 [[RW2, 128], [128 * RW2, BPS], [1, SLW]]))

            # gather all 64 chunks: one dma_gather per source shard
            hs = pg.tile([128, K * ROWP], DT.bfloat16, tag="hs")
            for sg in range(8):
                nc.gpsimd.dma_gather(
                    _ap3(hs, 8, ROWP, off=sg * 8 * ROWP),
                    tfull[sg * NPCT:(sg + 1) * NPCT, :],
                    idx[:, sg * 64:(sg + 1) * 64], 1024, 1024, ROWP,
                    queue_num=sg % 4)

            # spos[e, (c,pos)] = (dl[e,c] == pos)
            spos = pw.tile([128, K * 128], DT.bfloat16, tag="spos")
            nc.vector.tensor_tensor(
                out=_brd(spos[:], [[128, K], [1, 128]]),
                in0=_brd(iotaF[:], [[0, K], [1, 128]]),
                in1=_brd(dl[:], [[1, K], [0, 128]]),
                op=mybir.AluOpType.is_equal)
            # sposT[(pos), (c,e)] = (dlT[c,e] == pos), via ones-matmul bcast
            sposT = pw.tile([128, K * 128], DT.bfloat16, tag="sposT")
            for g in range(16):
                pT = psT.tile([128, 512], DT.float32, tag="pT")
                nc.tensor.matmul(pT[:], lhsT=ones1[:],
                                 rhs=dlT[:, g * 512:(g + 1) * 512],
                                 start=True, stop=True)
                nc.vector.tensor_tensor(
                    out=_brd(sposT[:], [[128, 4], [1, 128]], off=g * 512),
                    in0=_brd(iotaP[:], [[0, 4], [0, 128]]),
                    in1=_brd(pT[:], [[128, 4], [1, 128]]),
                    op=mybir.AluOpType.is_equal)

            # eadst via TensorE + e = asrc + eadst; leakyrelu; w = exp(e)
            e = pw.tile([128, K * nh], DT.float32, tag="e")
            for g8 in range(8):
                pE = psE.tile([128, 8 * nh], DT.float32, tag="pE")
                for j in range(8):
                    c = g8 * 8 + j
                    b = (c % 8) // SPB
                    nc.tensor.matmul(
                        pE[:, j * nh:(j + 1) * nh],
                        lhsT=sposT[:, c * 128:(c + 1) * 128],
                        rhs=ad[:, b * nh:(b + 1) * nh],
                        start=True, stop=True, skip_group_check=True)
                nc.vector.tensor_tensor(
                    out=_brd(e[:], [[nh, 8], [1, nh]], off=g8 * 8 * nh),
                    in0=_brd(hs[:], [[ROWP, 8], [1, nh]],
                             off=g8 * 8 * ROWP + asrc_c),
                    in1=_brd(pE[:], [[nh, 8], [1, nh]]),
                    op=mybir.AluOpType.add)
            tmp = pw.tile([128, K * nh], DT.float32, tag="etmp")
            nc.vector.tensor_scalar_mul(tmp[:], e[:], NEG)
            nc.vector.tensor_tensor(out=e[:], in0=e[:], in1=tmp[:],
                                    op=mybir.AluOpType.max)
            w = pw.tile([128, K * nh], DT.bfloat16, tag="w")
            nc.scalar.activation(w[:], e[:], mybir.ActivationFunctionType.Exp)

            # hw = [h*w | w]
            hw = pw.tile([128, K * nw], DT.bfloat16, tag="hw")
            if layer == 1:
                nc.vector.tensor_tensor(
                    out=_brd(hw[:], [[nw, K], [8, 8], [1, 8]]),
                    in0=_brd(hs[:], [[ROWP, K], [8, 8], [1, 8]]),
                    in1=_brd(w[:], [[nh, K], [1, 8], [0, 8]]),
                    op=mybir.AluOpType.mult)
                nc.vector.tensor_copy(
                    _brd(hw[:], [[nw, K], [1, 8]], off=64), w[:])
            else:
                nc.vector.tensor_tensor(
                    out=_brd(hw[:], [[nw, K], [1, C2]]),
                    in0=_brd(hs[:], [[ROWP, K], [1, C2]]),
                    in1=_brd(w[:], [[1, K], [0, C2]]),
                    op=mybir.AluOpType.mult)
                nc.vector.tensor_copy(
                    _brd(hw[:], [[nw, K], [1, 1]], off=C2), w[:])

            # per block: 16 accumulating matmuls + post
            for b in range(BPS):
                blk = psB.tile([128, nw], DT.float32, tag="blk")
                for q in range(16):
                    c = (q // 2) * 8 + b * SPB + (q % 2)
                    nc.tensor.matmul(
                        blk[:], lhsT=spos[:, c * 128:(c + 1) * 128],
                        rhs=hw[:, c * nw:(c + 1) * nw],
                        start=(q == 0), stop=(q == 15))
                asl = adsl[:, b * SLW:(b + 1) * SLW]
                if layer == 1:
                    _post1(nc, s, b, blk, asl, pb, psP, rwi, t2loc, ad2pos,
                           id_b, bias_sb, rhs2, state)
                else:
                    _post2(nc, s, b, blk, asl, pb, out_d, bias_sb)


def _post1(nc, s, b, blk, asl, pb, psP, rwi, t2loc, ad2pos, id_b, b1sb,
           rhs2, state):
    """Finalize one 128-position block of layer 1, emit table-2 rows.

    asl: [128, 80] position-aligned [h(64)|asrc(8)|adst(8)] rows -- used to
    fold each node's self-loop into the softmax analytically."""
    if b == 0:
        state["st2w"] = pb.tile([128, BPS * ROWP], DT.bfloat16, tag="st2w",
                                name="st2w")
        nc.vector.memset(state["st2w"][:], 0.0)
        state["staw"] = pb.tile([128, BPS * 64], DT.bfloat16, tag="staw",
                                name="staw")
        nc.vector.memset(state["staw"][:], 0.0)
    esl = pb.tile([128, 8], DT.float32, tag="esl")
    nc.vector.tensor_tensor(out=esl[:], in0=asl[:, 64:72], in1=asl[:, 72:80],
                            op=mybir.AluOpType.add)
    tsl = pb.tile([128, 8], DT.float32, tag="tsl")
    nc.vector.tensor_scalar_mul(tsl[:], esl[:], NEG)
    nc.vector.tensor_tensor(out=esl[:], in0=esl[:], in1=tsl[:],
                            op=mybir.AluOpType.max)
    wsl = pb.tile([128, 8], DT.float32, tag="wsl")
    nc.scalar.activation(wsl[:], esl[:], mybir.ActivationFunctionType.Exp)
    den = pb.tile([128, 8], DT.float32, tag="den")
    nc.vector.tensor_tensor(out=den[:], in0=blk[:, 64:72], in1=wsl[:],
                            op=mybir.AluOpType.add)
    nc.vector.tensor_scalar_max(den[:], den[:], 1e-30)
    rec = pb.tile([128, 8], DT.float32, tag="rec")
    nc.vector.reciprocal(rec[:], den[:])
    num = pb.tile([128, 64], DT.float32, tag="num")
    nc.vector.tensor_tensor(out=num[:], in0=asl[:, 0:64],
                            in1=_brd(wsl[:], [[1, 8], [0, 8]]),
                            op=mybir.AluOpType.mult)
    nc.vector.tensor_tensor(out=num[:], in0=num[:], in1=blk[:, 0:64],
                            op=mybir.AluOpType.add)
    hin = pb.tile([128, 64], DT.float32, tag="hin")
    nc.vector.tensor_tensor(
        out=hin[:], in0=num[:],
        in1=_brd(rec[:], [[1, 8], [0, 8]]), op=mybir.AluOpType.mult)
    nc.vector.tensor_tensor(out=hin[:], in0=hin[:], in1=b1sb[:],
                            op=mybir.AluOpType.add)
    # ELU = max(x,0) + exp(min(x,0)) - 1
    emn = pb.tile([128, 64], DT.float32, tag="emn")
    nc.vector.tensor_scalar_min(emn[:], hin[:], 0.0)
    nc.scalar.activation(emn[:], emn[:], mybir.ActivationFunctionType.Exp)
    nc.vector.tensor_scalar_max(hin[:], hin[:], 0.0)
    nc.vector.tensor_tensor(out=hin[:], in0=hin[:], in1=emn[:],
                            op=mybir.AluOpType.add)
    helu = pb.tile([128, 64], DT.bfloat16, tag="helu")
    nc.vector.tensor_scalar_add(helu[:], hin[:], -1.0)
    htp = psP.tile([64, 128], DT.bfloat16, tag="htp")
    nc.tensor.transpose(htp[:], helu[:], id_b[:])
    hts = pb.tile([64, 128], DT.bfloat16, tag="hts")
    nc.scalar.copy(hts[:], htp[:])
    h2ps = psP.tile([128, 42], DT.float32, tag="h2ps")
    nc.tensor.matmul(h2ps[:], lhsT=hts[:], rhs=rhs2[:], start=True, stop=True)
    nc.scalar.copy(state["st2w"][:, b * ROWP:b * ROWP + ROW2],
                   h2ps[:, 0:ROW2])
    nc.scalar.copy(state["staw"][:, b * 64:b * 64 + 42], h2ps[:, 0:42])
    if b == BPS - 1:
        nc.gpsimd.dma_scatter_add(
            t2loc, _ap3(state["st2w"], BPS, ROWP), rwi[:], 512, 512, ROWP,
            queue_num=s % 4)
        nc.sync.dma_start(
            bass.AP(ad2pos.tensor, s * BPS * 128 * 64,
                    [[64, 128], [128 * 64, BPS], [1, 64]]),
            state["staw"][:])


def _post2(nc, s, b, blk, asl, pb, out_d, b2sb):
    """asl: [128, 42] position-aligned [h2(40)|asrc2(1)|adst2(1)] rows."""
    esl = pb.tile([128, 1], DT.float32, tag="esl2")
    nc.vector.tensor_tensor(out=esl[:], in0=asl[:, 40:41], in1=asl[:, 41:42],
                            op=mybir.AluOpType.add)
    tsl = pb.tile([128, 1], DT.float32, tag="tsl2")
    nc.vector.tensor_scalar_mul(tsl[:], esl[:], NEG)
    nc.vector.tensor_tensor(out=esl[:], in0=esl[:], in1=tsl[:],
                            op=mybir.AluOpType.max)
    wsl = pb.tile([128, 1], DT.float32, tag="wsl2")
    nc.scalar.activation(wsl[:], esl[:], mybir.ActivationFunctionType.Exp)
    den = pb.tile([128, 1], DT.float32, tag="den2")
    nc.vector.tensor_tensor(out=den[:], in0=blk[:, C2:C2 + 1], in1=wsl[:],
                            op=mybir.AluOpType.add)
    nc.vector.tensor_scalar_max(den[:], den[:], 1e-30)
    rec = pb.tile([128, 1], DT.float32, tag="rec2")
    nc.vector.reciprocal(rec[:], den[:])
    num = pb.tile([128, C2], DT.float32, tag="num2")
    nc.vector.tensor_scalar(out=num[:], in0=asl[:, 0:C2], scalar1=wsl[:],
                            scalar2=None, op0=mybir.AluOpType.mult)
    nc.vector.tensor_tensor(out=num[:], in0=num[:], in1=blk[:, 0:C2],
                            op=mybir.AluOpType.add)
    o2 = pb.tile([128, C2], DT.float32, tag="o2")
    nc.vector.tensor_scalar(out=o2[:], in0=num[:], scalar1=rec[:],
                            scalar2=None, op0=mybir.AluOpType.mult)
    nc.vector.tensor_tensor(out=o2[:], in0=o2[:], in1=b2sb[:],
                            op=mybir.AluOpType.add)
    mx = pb.tile([128, 1], DT.float32, tag="mx")
    nc.vector.tensor_reduce(mx[:], o2[:], axis=mybir.AxisListType.X,
                            op=mybir.AluOpType.max)
    z = pb.tile([128, C2], DT.float32, tag="z")
    nc.vector.tensor_scalar(out=z[:], in0=o2[:], scalar1=mx[:], scalar2=None,
                            op0=mybir.AluOpType.subtract)
    ez = pb.tile([128, C2], DT.float32, tag="ez")
    se = pb.tile([128, 1], DT.float32, tag="se")
    nc.scalar.activation(ez[:], z[:], mybir.ActivationFunctionType.Exp,
                         accum_out=se[:])
    lse = pb.tile([128, 1], DT.float32, tag="lse")
    nc.scalar.activation(lse[:], se[:], mybir.ActivationFunctionType.Ln)
    zo = pb.tile([128, C2], DT.float32, tag="zo")
    nc.vector.tensor_scalar(out=zo[:], in0=z[:], scalar1=lse[:], scalar2=None,
                            op0=mybir.AluOpType.subtract)
    blkpos = (s * BPS + b) * 128
    nc.sync.dma_start(out_d[blkpos:blkpos + 128, :], zo[:])


# =================== SPMD runner (bass2jax-based, with timing) ===================

def _run_spmd(nc, in_maps, n_timing_iters=0):
    """Execute the program on NCORES neuron devices via PJRT (axon)."""
    import jax
    from jax.sharding import Mesh, PartitionSpec
    from jax.experimental.shard_map import shard_map
    from concourse import bass2jax
    from concourse.bass2jax import _bass_exec_p, partition_id_tensor
    import time

    bass2jax.install_neuronx_cc_hook()
    assert nc.dbg_addr is None or not nc.dbg_callbacks

    in_names, out_names, out_avals, zero_outs = [], [], [], []
    partition_name = (nc.partition_id_tensor.name
                      if nc.partition_id_tensor else None)
    for alloc in nc.m.functions[0].allocations:
        if not isinstance(alloc, mybir.MemoryLocationSet):
            continue
        name = alloc.memorylocations[0].name
        if alloc.kind == "ExternalInput":
            if name != partition_name:
                in_names.append(name)
        elif alloc.kind == "ExternalOutput":
            out_names.append(name)
            shape = tuple(alloc.tensor_shape)
            dtype = mybir.dt.np(alloc.dtype)
            out_avals.append(jax.core.ShapedArray(shape, dtype))
            zero_outs.append(np.zeros(shape, dtype))
    n_params = len(in_names)
    all_in_names = in_names + out_names + (
        [partition_name] if partition_name else [])

    def _body(*args):
        operands = list(args)
        if partition_name is not None:
            operands.append(partition_id_tensor())
        return tuple(_bass_exec_p.bind(
            *operands,
            out_avals=tuple(out_avals),
            in_names=tuple(all_in_names),
            out_names=tuple(out_names),
            lowering_input_output_aliases=(),
            sim_require_finite=True,
            sim_require_nnan=True,
            nc=nc,
        ))

    devices = jax.devices()[:NCORES]
    mesh = Mesh(np.asarray(devices), ("core",))
    nin = n_params + len(out_names)
    fn = jax.jit(shard_map(_body, mesh=mesh,
                           in_specs=(PartitionSpec("core"),) * nin,
                           out_specs=(PartitionSpec("core"),) * len(out_names),
                           check_rep=False),
                 keep_unused=True)
    sh = jax.sharding.NamedSharding(mesh, PartitionSpec("core"))
    concat_in = [
        jax.device_put(np.concatenate(
            [np.asarray(in_maps[c][name]) for c in range(NCORES)], axis=0), sh)
        for name in in_names
    ]
    concat_zeros = [
        jax.device_put(np.zeros((NCORES * z.shape[0], *z.shape[1:]), z.dtype),
                       sh) for z in zero_outs
    ]
    out_arrs = jax.block_until_ready(fn(*concat_in, *concat_zeros))
    times = []
    for _ in range(n_timing_iters):
        t0 = time.perf_counter()
        r = jax.block_until_ready(fn(*concat_in, *concat_zeros))
        times.append(time.perf_counter() - t0)
        del r
    results = [
        {name: np.asarray(out_arrs[i]).reshape(NCORES, *out_avals[i].shape)[c]
         for i, name in enumerate(out_names)}
        for c in range(NCORES)
    ]
    return results, times


# =================== top-level entry ===================

def kernel(**inputs):
    edge_index = np.asarray(inputs["edge_index"])
    meta, per_core = preprocess(edge_index)
    wts = build_weight_inputs(
        np.asarray(inputs["W1"]), np.asarray(inputs["att_src1"]),
        np.asarray(inputs["att_dst1"]), np.asarray(inputs["bias1"]),
        np.asarray(inputs["W2"]), np.asarray(inputs["att_src2"]),
        np.asarray(inputs["att_dst2"]), np.asarray(inputs["bias2"]))
    x = np.asarray(inputs["x"], _f32)
    in_maps = []
    for c in range(NCORES):
        xs = np.zeros((NP1T * 128, F), BF16)
        xs[:NPC] = x[c * NPC:(c + 1) * NPC].astype(BF16)
        in_maps.append(dict(
            x_sl=xs, W1=wts["W1"], A1=wts["A1"], W2=wts["W2"],
            att2=wts["att2"], b1r=wts["b1r"], b2r=wts["b2r"],
            idxw=per_core[c]["idxw"], dl=per_core[c]["dl"],
            dlT=per_core[c]["dlT"], rowiw=per_core[c]["rowiw"],
            pscatw=per_core[c]["pscatw"]))
    nc = build_program(meta)
    n_iters = int(os.environ.get("GAT_BENCH_ITERS", "0"))
    results, times = _run_spmd(nc, in_maps, n_timing_iters=n_iters)
    global LAST_TIMES
    LAST_TIMES = times
    out = np.zeros((N, C2), _f32)
    for c in range(NCORES):
        pm = per_core[c]["posmap_flat"]
        real = pm >= 0
        out[c * NPC + pm[real]] = results[c]["out"][np.nonzero(real)[0]]
    return out


# revision 32
# speedup vs baseline: 2.2347x; 2.2347x over previous
"""GAT (2-layer, PyG-style) distributed Bass kernel for 8 TRN2 NeuronCores.

Strategy (sharding_hint: 1D node partition by dst), v3:
  - core c owns dst nodes [c*NPC, (c+1)*NPC).
  - dense phase: each core computes table1 rows [h1(64)|a_src1(8)|pad->256B]
    for its node slice; AllGather -> full padded table in every core's DRAM.
    A position-ordered copy [h|asrc|adst] (ad1pos) is written in parallel by
    dma_scatter_add (1024 int16 indices/call; rows unique so add==write on
    the zeroed table).
  - edge phase: self-loop edges are NOT materialized (handled analytically in
    the post-pass, position-aligned).  Remaining edges: host packs dst nodes
    into BLOCKS of <=128 nodes whose edges, split by SOURCE SHARD, fit 2
    single-shard chunks of 128 slots per shard (8-dim FFD; balanced because
    self-loops no longer skew the local shard).  4 blocks = 1 supertile = 64
    chunks, chunk c = shard*8 + block_loc*2 + half.  Per supertile:
      * 8 dma_gather calls (Q7 custom op, 1024 int16 shard-local indices,
        256B rows, spread over 4 SWDGE queues -> ~0.9us/call) pull all 8192
        edge source rows.  vs ~1us per 128-row indirect DMA = ~9x fewer
        Pool-engine descriptor-generation stalls.
      * per-edge a_dst: eadst[e,h] = sposT_chunk^T @ ad_block on TensorE,
        where sposT (pos->edge one-hot) is built by iota/is_equal from a
        K=1 ones-matmul broadcast of the dst-position vector dloc.
      * w = exp(leakyrelu(a_src+a_dst)) (no max subtraction; |logit| < ~3),
        hw = [h*w | w], then per block 16 accumulating matmuls
        blk[128pos, nw] += spos_chunk^T @ hw_chunk (PSUM).
  - block post: fold in the self-loop term (w_sl = exp(lrelu(asrc+adst)),
    num += w_sl*h, den += w_sl) from the position-ordered table, normalize,
    +bias, ELU, W2aug matmul -> layer-2 node rows (dma_scatter_add into the
    zeroed padded table) + position-ordered copy; AllGather #2; same edge
    pass for layer 2; log_softmax; output rows in position order (host
    unpacks by posmap).
All floating-point math runs on-device.  Host work is integer graph
preprocessing (sort/pack/index building) and weight layout rearrangement.
"""
import os
import sys
import numpy as np

try:
    import concourse.bass as bass
except ImportError:  # pragma: no cover
    for p in ("/opt/trn_rl_repo", "/root/.axon_site/_ro/trn_rl_repo"):
        if os.path.isdir(p) and p not in sys.path:
            sys.path.insert(0, p)
    import concourse.bass as bass

import ml_dtypes
import concourse.mybir as mybir
import concourse.tile as tile
import concourse.bacc as bacc
from concourse import library_config
from concourse.masks import make_identity

BF16 = ml_dtypes.bfloat16
DT = mybir.dt

# ---------------- problem config (hardcoded per contract) ----------------
N, E, F = 100000, 1600000, 256
H1, C1 = 8, 8          # layer1 heads x channels (concat -> 64)
C2 = 40                # layer2 single head, 40 classes
NEG = 0.2
NCORES = 8
NPC = N // NCORES      # 12500
NPCT = 12672           # table rows per shard (12544 P1-padded + 128 dump)
ROWP = 128             # padded table row, bf16 elements (= 256B)
TILE_E = 128           # edge slots per tile/chunk
SPB = 2                # chunks per (block, shard)
BPS = 4                # blocks per supertile
K = 64                 # chunks per supertile = 8 shards x 4 blocks x 2
ROW1 = 72              # useful cols layer1: [h1(64) | asrc1(8)]
ROW2 = 41              # useful cols layer2: [h2(40) | asrc2(1)]
NP1T = 99              # P1 tiles (12672 rows / 128)
NP1G = 13              # P1 scatter groups of 8 tiles

_f32 = np.float32


# =================== host-side graph preprocessing ===================

def _pack_blocks(deg2):
    """8-dim FFD: pack nodes into blocks (<=128 nodes, per-shard edge count
    <=SPB*TILE_E).  deg2: [NPC, 8] per-node per-source-shard edge counts."""
    cap = SPB * TILE_E
    order = np.argsort(-deg2.sum(1), kind="stable")
    blocks = []
    rem = np.zeros((0, 8), np.int64)
    npos = np.zeros(0, np.int64)
    open_ids = np.zeros(0, np.int64)
    for n in order:
        dn = deg2[n]
        ok = np.nonzero((rem >= dn).all(1) & (npos < 128))[0]
        if ok.size:
            k = ok[0]
            bi = open_ids[k]
            blocks[bi].append(int(n))
            rem[k] -= dn
            npos[k] += 1
            if npos[k] >= 128 or rem[k].sum() < 3:
                keep = np.arange(rem.shape[0]) != k
                rem, npos, open_ids = rem[keep], npos[keep], open_ids[keep]
        else:
            blocks.append([int(n)])
            rem = np.vstack([rem, (cap - dn)[None]])
            npos = np.append(npos, 1)
            open_ids = np.append(open_ids, len(blocks) - 1)
        if rem.shape[0] > 64:
            drop = np.argmin(rem.sum(1))
            keep = np.arange(rem.shape[0]) != drop
            rem, npos, open_ids = rem[keep], npos[keep], open_ids[keep]
    return blocks


def _wrap16(vals):
    """[n] -> wrapped int16 [128, n/16] layout: index i at [i%16, i//16],
    replicated across the 8 groups of 16 partitions."""
    n = vals.shape[-1]
    v = vals.reshape(*vals.shape[:-1], n // 16, 16)
    v = np.moveaxis(v, -1, -2)  # [..., 16, n//16]
    v = np.concatenate([v] * 8, axis=-2)  # tile to 128 partitions
    return np.ascontiguousarray(v).astype(np.int16)


def preprocess(edge_index):
    # self-loops are handled analytically in the post-pass (position-aligned)
    src = edge_index[0].astype(np.int64)
    dst = edge_index[1].astype(np.int64)

    cores = []
    max_nb = 0
    for c in range(NCORES):
        lo = c * NPC
        m = (dst >= lo) & (dst < lo + NPC)
        s_c, d_c = src[m], dst[m] - lo
        shard = s_c // NPC
        srow = s_c % NPC
        key = d_c * 8 + shard
        order = np.argsort(key, kind="stable")
        srow_s = srow[order]
        deg2 = np.bincount(key, minlength=NPC * 8).reshape(NPC, 8)
        starts = np.zeros(NPC * 8 + 1, np.int64)
        starts[1:] = np.cumsum(deg2.reshape(-1))
        blocks = _pack_blocks(deg2)
        cores.append(dict(srow_s=srow_s, starts=starts, blocks=blocks))
        max_nb = max(max_nb, len(blocks))

    nblocks = -(-max_nb // BPS) * BPS
    assert nblocks * 128 + 128 <= 32767, f"NPOS overflow: {nblocks}"
    S = nblocks // BPS
    NPOS = nblocks * 128
    DUMP = 12544       # dump row base for t2loc (rows [12544, 12672))

    per_core = []
    for cd in cores:
        blocks, starts, srow_s = cd["blocks"], cd["starts"], cd["srow_s"]
        blocks = blocks + [[] for _ in range(nblocks - len(blocks))]
        posmap = np.full((nblocks, 128), -1, np.int64)
        # node/pad -> position; pads & tail -> spread dump positions
        pscat = NPOS + (np.arange(NP1G * 8 * 128, dtype=np.int64) % 128)
        idx_loc = np.zeros((nblocks, 8, SPB * TILE_E), np.int64)
        dpos = np.full((nblocks, 8, SPB * TILE_E), 255, np.int64)
        for b, nodes in enumerate(blocks):
            ptr = np.zeros(8, np.int64)
            for pos, n in enumerate(nodes):
                posmap[b, pos] = n
                pscat[n] = b * 128 + pos
                for sg in range(8):
                    a, z = starts[n * 8 + sg], starts[n * 8 + sg + 1]
                    k = z - a
                    if k:
                        p0 = ptr[sg]
                        idx_loc[b, sg, p0:p0 + k] = srow_s[a:z]
                        dpos[b, sg, p0:p0 + k] = pos
                        ptr[sg] += k
            assert (ptr <= SPB * TILE_E).all()
        # chunk layout: global chunk c = sg*8 + b_loc*SPB + half; call sg
        # covers chunks [sg*8, sg*8+8) = its supertile's 4 blocks x 2 halves
        idx4 = idx_loc.reshape(S, BPS, 8, SPB * TILE_E).transpose(0, 2, 1, 3)
        idxw = _wrap16(idx4.reshape(S, 8, 8 * TILE_E)).reshape(S, 8, 128, 64)
        idxw = np.ascontiguousarray(idxw.transpose(0, 2, 1, 3)).reshape(
            S, 128, 512)
        # dl [S, 128slot, 64chunk]
        dlc = dpos.reshape(S, BPS, 8, SPB, TILE_E).transpose(0, 2, 1, 3, 4)
        dl = np.ascontiguousarray(
            dlc.reshape(S, K, TILE_E).transpose(0, 2, 1)).astype(BF16)
        # dlT [S, 1, 64*128]: [0, c*128 + slot] = dloc(chunk c, slot)
        dlT = np.ascontiguousarray(
            dlc.reshape(S, 1, K * TILE_E)).astype(BF16)
        # rowiw [S, 128, 32]: scatter idx for post blocks: i=b_loc*128+pos
        rowv = posmap.reshape(S, BPS * 128).copy()
        bad = rowv < 0
        rowv[bad] = DUMP + (np.nonzero(bad)[1] % 128)
        rowiw = _wrap16(rowv)
        # pscatw [NP1G, 128, 64]: P1 tile j covers nodes j*128..j*128+127
        pv = pscat[:NP1G * 8 * 128].reshape(NP1G, 8 * 128)
        pscatw = _wrap16(pv)
        per_core.append(dict(idxw=idxw, dl=dl, dlT=dlT, rowiw=rowiw,
                             pscatw=pscatw,
                             posmap_flat=posmap.reshape(-1).copy()))
    meta = dict(S=S, nblocks=nblocks, NPOS=NPOS)
    return meta, per_core


def build_weight_inputs(W1, att_src1, att_dst1, bias1, W2, att_src2, att_dst2,
                        bias2):
    """Pure layout rearrangement of weights (no FP arithmetic)."""
    A1 = np.zeros((64, 16), _f32)
    for h in range(H1):
        A1[h * 8:(h + 1) * 8, h] = att_src1[h]
        A1[h * 8:(h + 1) * 8, 8 + h] = att_dst1[h]
    att2 = np.concatenate([att_src2.T, att_dst2.T], axis=1).astype(_f32)
    b1r = np.broadcast_to(bias1.astype(_f32), (128, 64)).copy()
    b2r = np.broadcast_to(bias2.astype(_f32), (128, C2)).copy()
    return dict(W1=W1.astype(_f32), A1=A1, W2=W2.astype(_f32), att2=att2,
                b1r=b1r, b2r=b2r)


# =================== device program ===================

def _brd(ap, pattern, off=0):
    """Manual AP: keep partition dim, explicit free-dim [step,count] pattern."""
    return bass.AP(ap.tensor, ap.offset + off, [ap.ap[0]] + pattern)


def _ap3(t, chunks, elem, off=0):
    a = t[:]
    return bass.AP(a.tensor, a.offset + off, [a.ap[0], [elem, chunks],
                                              [1, elem]])


def build_program(meta):
    S, NPOS = meta["S"], meta["NPOS"]
    NT = NPCT * NCORES

    nc = bacc.Bacc("TRN2", target_bir_lowering=False, debug=False,
                   enable_asserts=False, num_devices=NCORES,
                   num_swdge_queues=4)

    def din(name, shape, dt):
        return nc.dram_tensor(name, shape, dt, kind="ExternalInput").ap()

    x_sl = din("x_sl", [NP1T * 128, F], DT.bfloat16)
    W1 = din("W1", [F, 64], DT.float32)
    A1 = din("A1", [64, 16], DT.float32)
    W2 = din("W2", [64, C2], DT.float32)
    att2 = din("att2", [C2, 2], DT.float32)
    b1r = din("b1r", [128, 64], DT.float32)
    b2r = din("b2r", [128, C2], DT.float32)
    idxw_d = din("idxw", [S, 128, 512], DT.int16)
    dl_d = din("dl", [S, 128, K], DT.bfloat16)
    dlT_d = din("dlT", [S, 1, K * 128], DT.bfloat16)
    rowiw_d = din("rowiw", [S, 128, 32], DT.int16)
    pscatw_d = din("pscatw", [NP1G, 128, 64], DT.int16)

    out_d = nc.dram_tensor("out", [NPOS, C2], DT.float32,
                           kind="ExternalOutput").ap()

    t1loc = nc.dram_tensor("t1loc", [NPCT, ROWP], DT.bfloat16).ap()
    t1full = nc.dram_tensor("t1full", [NT, ROWP], DT.bfloat16,
                            addr_space="Shared").ap()
    t2loc = nc.dram_tensor("t2loc", [NPCT, ROWP], DT.bfloat16).ap()
    t2full = nc.dram_tensor("t2full", [NT, ROWP], DT.bfloat16,
                            addr_space="Shared").ap()
    ad1pos = nc.dram_tensor("ad1pos", [NPOS + 128, ROWP], DT.bfloat16).ap()
    ad2pos = nc.dram_tensor("ad2pos", [NPOS, 64], DT.bfloat16).ap()

    groups = [list(range(NCORES))]

    with tile.TileContext(nc, num_cores=NCORES) as tc:
        from contextlib import ExitStack
        with ExitStack() as top:
            cpool = top.enter_context(tc.tile_pool(name="const", bufs=1))
            id_f = cpool.tile([128, 128], DT.float32)
            make_identity(nc, id_f[:])
            id_b = cpool.tile([128, 128], DT.bfloat16)
            nc.vector.tensor_copy(id_b[:], id_f[:])
            ioF_i = cpool.tile([128, 128], DT.int16)
            nc.gpsimd.iota(ioF_i[:], pattern=[[1, 128]], base=0,
                           channel_multiplier=0)
            iotaF = cpool.tile([128, 128], DT.bfloat16)
            nc.vector.tensor_copy(iotaF[:], ioF_i[:])
            ioP_i = cpool.tile([128, 1], DT.int16)
            nc.gpsimd.iota(ioP_i[:], pattern=[[0, 1]], base=0,
                           channel_multiplier=1)
            iotaP = cpool.tile([128, 1], DT.bfloat16)
            nc.vector.tensor_copy(iotaP[:], ioP_i[:])
            ones1 = cpool.tile([1, 128], DT.bfloat16)
            nc.vector.memset(ones1[:], 1.0)
            b1sb = cpool.tile([128, 64], DT.float32)
            nc.sync.dma_start(b1sb[:], b1r)
            b2sb = cpool.tile([128, C2], DT.float32)
            nc.sync.dma_start(b2sb[:], b2r)
            # switch Q7 library: enables dma_gather / dma_scatter_add
            nc.gpsimd.load_library(library_config.mlp)

            # ---------- P0: weight prep ----------
            rhs1 = [cpool.tile([128, 80], DT.bfloat16, tag=f"rhs1_{i}",
                               name=f"rhs1_{i}") for i in range(2)]
            rhs2 = cpool.tile([64, 42], DT.bfloat16)
            with tc.tile_pool(name="p0", bufs=1) as p0, \
                 tc.tile_pool(name="p0ps", bufs=1, space="PSUM") as p0ps:
                w1sb = [p0.tile([128, 64], DT.float32, tag=f"w1_{i}",
                                name=f"w1_{i}") for i in range(2)]
                for i in range(2):
                    nc.sync.dma_start(w1sb[i][:], W1[128 * i:128 * (i + 1), :])
                a1sb = p0.tile([64, 16], DT.float32)
                nc.sync.dma_start(a1sb[:], A1)
                w2sb = p0.tile([64, C2], DT.float32)
                nc.sync.dma_start(w2sb[:], W2)
                at2sb = p0.tile([C2, 2], DT.float32)
                nc.sync.dma_start(at2sb[:], att2)
                for i in range(2):
                    tp = p0ps.tile([64, 128], DT.float32, tag="w1t_ps")
                    nc.tensor.transpose(tp[:], w1sb[i][:], id_f[:])
                    w1t = p0.tile([64, 128], DT.float32, tag="w1t")
                    nc.vector.tensor_copy(w1t[:], tp[:])
                    wa = p0ps.tile([128, 16], DT.float32, tag="w1a_ps")
                    nc.tensor.matmul(wa[:], lhsT=w1t[:], rhs=a1sb[:],
                                     start=True, stop=True)
                    nc.vector.tensor_copy(rhs1[i][:, 0:64], w1sb[i][:])
                    nc.vector.tensor_copy(rhs1[i][:, 64:80], wa[:])
                tp2 = p0ps.tile([C2, 64], DT.float32, tag="w2t_ps")
                nc.tensor.transpose(tp2[:], w2sb[:], id_f[:64, :64])
                w2t = p0.tile([C2, 64], DT.float32)
                nc.vector.tensor_copy(w2t[:], tp2[:])
                wa2 = p0ps.tile([64, 2], DT.float32, tag="w2a_ps")
                nc.tensor.matmul(wa2[:], lhsT=w2t[:], rhs=at2sb[:],
                                 start=True, stop=True)
                nc.vector.tensor_copy(rhs2[:, 0:C2], w2sb[:])
                nc.vector.tensor_copy(rhs2[:, C2:C2 + 2], wa2[:])

            # ---------- P0.5: zero scatter-add target tables ----------
            with tc.tile_pool(name="pz", bufs=1) as pz:
                ZW = 4096
                zt = pz.tile([128, ZW], DT.bfloat16)
                nc.vector.memset(zt[:], 0.0)
                for tgt, nelem in ((ad1pos, (NPOS + 128) * ROWP),
                                   (ad2pos, NPOS * 64),
                                   (t2loc, NPCT * ROWP)):
                    done = 0
                    while done < nelem:
                        chunk = min(ZW * 128, nelem - done)
                        w = chunk // 128
                        nc.sync.dma_start(
                            bass.AP(tgt.tensor, done, [[w, 128], [1, w]]),
                            zt[:, 0:w])
                        done += w * 128

            # ---------- P1: dense layer-1 table ----------
            with tc.tile_pool(name="p1", bufs=3) as p1, \
                 tc.tile_pool(name="p1ps", bufs=2, space="PSUM") as p1ps:
                sta8 = opw = None
                for it in range(NP1T):
                    g, gi = divmod(it, 8)
                    if gi == 0:
                        sta8 = p1.tile([128, 8 * ROWP], DT.bfloat16,
                                       tag="sta8", name="sta8")
                        nc.vector.memset(sta8[:], 0.0)
                        opw = p1.tile([128, 64], DT.int16, tag="opw")
                        nc.sync.dma_start(opw[:], pscatw_d[g])
                    xb = p1.tile([128, F], DT.bfloat16, tag="xb")
                    nc.sync.dma_start(xb[:], x_sl[128 * it:128 * (it + 1), :])
                    xT = p1.tile([128, F], DT.bfloat16, tag="xT")
                    ps1 = p1ps.tile([128, 80], DT.float32, tag="ps1")
                    for i in range(2):
                        tp = p1ps.tile([128, 128], DT.bfloat16, tag="xt_ps")
                        nc.tensor.transpose(
                            tp[:], xb[:, 128 * i:128 * (i + 1)], id_b[:])
                        nc.scalar.copy(xT[:, 128 * i:128 * (i + 1)], tp[:])
                    for i in range(2):
                        nc.tensor.matmul(
                            ps1[:], lhsT=xT[:, 128 * i:128 * (i + 1)],
                            rhs=rhs1[i][:], start=(i == 0), stop=(i == 1))
                    st = p1.tile([128, ROWP], DT.bfloat16, tag="st1")
                    nc.scalar.copy(st[:, 0:ROW1], ps1[:, 0:ROW1])
                    nc.scalar.copy(sta8[:, ROWP * gi:ROWP * gi + 80],
                                   ps1[:, 0:80])
                    nc.sync.dma_start(t1loc[128 * it:128 * (it + 1), :], st[:])
                    if gi == 7 or it == NP1T - 1:
                        nc.gpsimd.dma_scatter_add(
                            ad1pos, _ap3(sta8, 8, ROWP), opw[:],
                            1024, 1024, ROWP, queue_num=g % 4)

            # ---------- P2: AllGather table1 ----------
            nc.gpsimd.collective_compute(
                "AllGather", mybir.AluOpType.bypass, replica_groups=groups,
                ins=[t1loc.opt()], outs=[t1full.opt()])

            # ---------- P3: edge pass layer 1 ----------
            edge_pass(nc, tc, meta, 1, idxw_d, dl_d, dlT_d, rowiw_d,
                      t1full, ad1pos, t2loc, ad2pos, None,
                      iotaF, iotaP, ones1, id_b, b1sb, rhs2)

            # ---------- P4: AllGather table2 ----------
            nc.gpsimd.collective_compute(
                "AllGather", mybir.AluOpType.bypass, replica_groups=groups,
                ins=[t2loc.opt()], outs=[t2full.opt()])

            # ---------- P5: edge pass layer 2 ----------
            edge_pass(nc, tc, meta, 2, idxw_d, dl_d, dlT_d, rowiw_d,
                      t2full, ad2pos, None, None, out_d,
                      iotaF, iotaP, ones1, id_b, b2sb, None)

    nc.compile()
    return nc


def edge_pass(nc, tc, meta, layer, idxw_d, dl_d, dlT_d, rowiw_d, tfull,
              adpos, t2loc, ad2pos, out_d, iotaF, iotaP, ones1, id_b,
              bias_sb, rhs2):
    S = meta["S"]
    nh = H1 if layer == 1 else 1          # heads
    nch = 64 if layer == 1 else C2        # message channels
    asrc_c = nch                          # a_src column in table row
    nw = nch + nh                         # hw width: [msgs*w | w]
    from contextlib import ExitStack
    with ExitStack() as ctx:
        pm = ctx.enter_context(tc.tile_pool(name=f"e{layer}m", bufs=2))
        pg = ctx.enter_context(tc.tile_pool(name=f"e{layer}g", bufs=2))
        pw = ctx.enter_context(tc.tile_pool(name=f"e{layer}w", bufs=2))
        pb = ctx.enter_context(tc.tile_pool(name=f"e{layer}b", bufs=2))
        psT = ctx.enter_context(
            tc.tile_pool(name=f"e{layer}pT", bufs=1, space="PSUM"))
        psE = ctx.enter_context(
            tc.tile_pool(name=f"e{layer}pE", bufs=2, space="PSUM"))
        psB = ctx.enter_context(
            tc.tile_pool(name=f"e{layer}pB", bufs=2, space="PSUM"))
        psP = ctx.enter_context(
            tc.tile_pool(name=f"e{layer}pP", bufs=1, space="PSUM"))
        for s in range(S):
            idx = pm.tile([128, 512], DT.int16, tag="idx")
            nc.sync.dma_start(idx[:], idxw_d[s])
            dl = pm.tile([128, K], DT.bfloat16, tag="dl")
            nc.sync.dma_start(dl[:], dl_d[s])
            dlT = pm.tile([1, K * 128], DT.bfloat16, tag="dlT")
            nc.sync.dma_start(dlT[:], dlT_d[s])
            rwi = pm.tile([128, 32], DT.int16, tag="rwi")
            if layer == 1:
                nc.sync.dma_start(rwi[:], rowiw_d[s])
            SLW = 80 if layer == 1 else 42
            RW2 = ROWP if layer == 1 else 64
            ad = pm.tile([128, BPS * nh], DT.bfloat16, tag="ad")
            adsl = pm.tile([128, BPS * SLW], DT.bfloat16, tag="adsl")
            adoff = 72 if layer == 1 else 41
            nc.sync.dma_start(
                ad[:], bass.AP(adpos.tensor, s * BPS * 128 * RW2 + adoff,
                               [[RW2, 128], [128 * RW2, BPS], [1, nh]]))
            nc.sync.dma_start(
                adsl[:], bass.AP(adpos.tensor, s * BPS * 128 * RW2,
                                 [[RW2, 128], [128 * RW2, BPS], [1, SLW]]))

            # gather all 64 chunks: one dma_gather per source shard
            hs = pg.tile([128, K * ROWP], DT.bfloat16, tag="hs")
            for sg in range(8):
                nc.gpsimd.dma_gather(
                    _ap3(hs, 8, ROWP, off=sg * 8 * ROWP),
                    tfull[sg * NPCT:(sg + 1) * NPCT, :],
                    idx[:, sg * 64:(sg + 1) * 64], 1024, 1024, ROWP,
                    queue_num=sg % 4)

            # spos[e, (c,pos)] = (dl[e,c] == pos)
            spos = pw.tile([128, K * 128], DT.bfloat16, tag="spos")
            nc.vector.tensor_tensor(
                out=_brd(spos[:], [[128, K], [1, 128]]),
                in0=_brd(iotaF[:], [[0, K], [1, 128]]),
                in1=_brd(dl[:], [[1, K], [0, 128]]),
                op=mybir.AluOpType.is_equal)
            # sposT[(pos), (c,e)] = (dlT[c,e] == pos), via ones-matmul bcast
            sposT = pw.tile([128, K * 128], DT.bfloat16, tag="sposT")
            for g in range(16):
                pT = psT.tile([128, 512], DT.float32, tag="pT")
                nc.tensor.matmul(pT[:], lhsT=ones1[:],
                                 rhs=dlT[:, g * 512:(g + 1) * 512],
                                 start=True, stop=True)
                nc.vector.tensor_tensor(
                    out=_brd(sposT[:], [[128, 4], [1, 128]], off=g * 512),
                    in0=_brd(iotaP[:], [[0, 4], [0, 128]]),
                    in1=_brd(pT[:], [[128, 4], [1, 128]]),
                    op=mybir.AluOpType.is_equal)

            # eadst via TensorE + e = asrc + eadst; leakyrelu; w = exp(e)
            e = pw.tile([128, K * nh], DT.float32, tag="e")
            for g8 in range(8):
                pE = psE.tile([128, 8 * nh], DT.float32, tag="pE")
                for j in range(8):
                    c = g8 * 8 + j
                    b = (c % 8) // SPB
                    nc.tensor.matmul(
                        pE[:, j * nh:(j + 1) * nh],
                        lhsT=sposT[:, c * 128:(c + 1) * 128],
                        rhs=ad[:, b * nh:(b + 1) * nh],
                        start=True, stop=True, skip_group_check=True)
                nc.vector.tensor_tensor(
                    out=_brd(e[:], [[nh, 8], [1, nh]], off=g8 * 8 * nh),
                    in0=_brd(hs[:], [[ROWP, 8], [1, nh]],
                             off=g8 * 8 * ROWP + asrc_c),
                    in1=_brd(pE[:], [[nh, 8], [1, nh]]),
                    op=mybir.AluOpType.add)
            tmp = pw.tile([128, K * nh], DT.float32, tag="etmp")
            nc.vector.tensor_scalar_mul(tmp[:], e[:], NEG)
            nc.vector.tensor_tensor(out=e[:], in0=e[:], in1=tmp[:],
                                    op=mybir.AluOpType.max)
            w = pw.tile([128, K * nh], DT.bfloat16, tag="w")
            nc.scalar.activation(w[:], e[:], mybir.ActivationFunctionType.Exp)

            # hw = [h*w | w]
            hw = pw.tile([128, K * nw], DT.bfloat16, tag="hw")
            if layer == 1:
                nc.vector.tensor_tensor(
                    out=_brd(hw[:], [[nw, K], [8, 8], [1, 8]]),
                    in0=_brd(hs[:], [[ROWP, K], [8, 8], [1, 8]]),
                    in1=_brd(w[:], [[nh, K], [1, 8], [0, 8]]),
                    op=mybir.AluOpType.mult)
                nc.vector.tensor_copy(
                    _brd(hw[:], [[nw, K], [1, 8]], off=64), w[:])
            else:
                nc.vector.tensor_tensor(
                    out=_brd(hw[:], [[nw, K], [1, C2]]),
                    in0=_brd(hs[:], [[ROWP, K], [1, C2]]),
                    in1=_brd(w[:], [[1, K], [0, C2]]),
                    op=mybir.AluOpType.mult)
                nc.vector.tensor_copy(
                    _brd(hw[:], [[nw, K], [1, 1]], off=C2), w[:])

            # per block: 16 accumulating matmuls into one wide PSUM tile
            blk = psB.tile([128, BPS * nw], DT.float32, tag="blk")
            for b in range(BPS):
                for q in range(16):
                    c = (q // 2) * 8 + b * SPB + (q % 2)
                    nc.tensor.matmul(
                        blk[:, b * nw:(b + 1) * nw],
                        lhsT=spos[:, c * 128:(c + 1) * 128],
                        rhs=hw[:, c * nw:(c + 1) * nw],
                        start=(q == 0), stop=(q == 15),
                        skip_group_check=True)
            if layer == 1:
                _post1(nc, s, blk, adsl, pb, psP, rwi, t2loc, ad2pos,
                       id_b, bias_sb, rhs2)
            else:
                _post2(nc, s, blk, adsl, pb, out_d, bias_sb)


def _post1(nc, s, blk, adsl, pb, psP, rwi, t2loc, ad2pos, id_b, b1sb,
           rhs2):
    """Finalize all 4 blocks of a supertile (layer 1), emit table-2 rows.

    blk:  [128, 4*72] PSUM, per block [msgs(64)|denoms(8)].
    adsl: [128, 4*80] position-aligned [h|asrc|adst] rows (self-loop fold).
    """
    SLW = 80
    nw = 72
    st2w = pb.tile([128, BPS * ROWP], DT.bfloat16, tag="st2w")
    nc.vector.memset(st2w[:], 0.0)
    staw = pb.tile([128, BPS * 64], DT.bfloat16, tag="staw")
    nc.vector.memset(staw[:], 0.0)
    esl = pb.tile([128, BPS * 8], DT.float32, tag="esl")
    nc.vector.tensor_tensor(
        out=_brd(esl[:], [[8, BPS], [1, 8]]),
        in0=_brd(adsl[:], [[SLW, BPS], [1, 8]], off=64),
        in1=_brd(adsl[:], [[SLW, BPS], [1, 8]], off=72),
        op=mybir.AluOpType.add)
    tsl = pb.tile([128, BPS * 8], DT.float32, tag="tsl")
    nc.vector.tensor_scalar_mul(tsl[:], esl[:], NEG)
    nc.vector.tensor_tensor(out=esl[:], in0=esl[:], in1=tsl[:],
                            op=mybir.AluOpType.max)
    wsl = pb.tile([128, BPS * 8], DT.float32, tag="wsl")
    nc.scalar.activation(wsl[:], esl[:], mybir.ActivationFunctionType.Exp)
    den = pb.tile([128, BPS * 8], DT.float32, tag="den")
    nc.vector.tensor_tensor(
        out=_brd(den[:], [[8, BPS], [1, 8]]),
        in0=_brd(blk[:], [[nw, BPS], [1, 8]], off=64),
        in1=_brd(wsl[:], [[8, BPS], [1, 8]]),
        op=mybir.AluOpType.add)
    nc.vector.tensor_scalar_max(den[:], den[:], 1e-30)
    rec = pb.tile([128, BPS * 8], DT.float32, tag="rec")
    nc.vector.reciprocal(rec[:], den[:])
    num = pb.tile([128, BPS * 64], DT.float32, tag="num")
    nc.vector.tensor_tensor(
        out=_brd(num[:], [[64, BPS], [8, 8], [1, 8]]),
        in0=_brd(adsl[:], [[SLW, BPS], [8, 8], [1, 8]]),
        in1=_brd(wsl[:], [[8, BPS], [1, 8], [0, 8]]),
        op=mybir.AluOpType.mult)
    nc.vector.tensor_tensor(
        out=_brd(num[:], [[64, BPS], [1, 64]]),
        in0=_brd(num[:], [[64, BPS], [1, 64]]),
        in1=_brd(blk[:], [[nw, BPS], [1, 64]]),
        op=mybir.AluOpType.add)
    hin = pb.tile([128, BPS * 64], DT.float32, tag="hin")
    nc.vector.tensor_tensor(
        out=_brd(hin[:], [[64, BPS], [8, 8], [1, 8]]),
        in0=_brd(num[:], [[64, BPS], [8, 8], [1, 8]]),
        in1=_brd(rec[:], [[8, BPS], [1, 8], [0, 8]]),
        op=mybir.AluOpType.mult)
    nc.vector.tensor_tensor(
        out=_brd(hin[:], [[64, BPS], [1, 64]]),
        in0=_brd(hin[:], [[64, BPS], [1, 64]]),
        in1=_brd(b1sb[:], [[0, BPS], [1, 64]]),
        op=mybir.AluOpType.add)
    # ELU = max(x,0) + exp(min(x,0)) - 1
    emn = pb.tile([128, BPS * 64], DT.float32, tag="emn")
    nc.vector.tensor_scalar_min(emn[:], hin[:], 0.0)
    nc.scalar.activation(emn[:], emn[:], mybir.ActivationFunctionType.Exp)
    nc.vector.tensor_scalar_max(hin[:], hin[:], 0.0)
    nc.vector.tensor_tensor(out=hin[:], in0=hin[:], in1=emn[:],
                            op=mybir.AluOpType.add)
    helu = pb.tile([128, BPS * 64], DT.bfloat16, tag="helu")
    nc.vector.tensor_scalar_add(helu[:], hin[:], -1.0)
    for b in range(BPS):
        htp = psP.tile([64, 128], DT.bfloat16, tag="htp")
        nc.tensor.transpose(htp[:], helu[:, b * 64:(b + 1) * 64], id_b[:])
        hts = pb.tile([64, 128], DT.bfloat16, tag="hts")
        nc.scalar.copy(hts[:], htp[:])
        h2ps = psP.tile([128, 42], DT.float32, tag="h2ps")
        nc.tensor.matmul(h2ps[:], lhsT=hts[:], rhs=rhs2[:], start=True,
                         stop=True)
        nc.scalar.copy(st2w[:, b * ROWP:b * ROWP + ROW2], h2ps[:, 0:ROW2])
        nc.scalar.copy(staw[:, b * 64:b * 64 + 42], h2ps[:, 0:42])
    nc.gpsimd.dma_scatter_add(
        t2loc, _ap3(st2w, BPS, ROWP), rwi[:], 512, 512, ROWP,
        queue_num=s % 4)
    nc.sync.dma_start(
        bass.AP(ad2pos.tensor, s * BPS * 128 * 64,
                [[64, 128], [128 * 64, BPS], [1, 64]]),
        staw[:])


def _post2(nc, s, blk, adsl, pb, out_d, b2sb):
    """Finalize all 4 blocks of a supertile (layer 2): log_softmax rows.

    blk:  [128, 4*41] PSUM, per block [msgs(40)|denom(1)].
    adsl: [128, 4*42] position-aligned [h2|asrc2|adst2] rows.
    """
    SLW = 42
    nw = C2 + 1
    esl = pb.tile([128, BPS], DT.float32, tag="esl2")
    nc.vector.tensor_tensor(
        out=_brd(esl[:], [[1, BPS], [1, 1]]),
        in0=_brd(adsl[:], [[SLW, BPS], [1, 1]], off=40),
        in1=_brd(adsl[:], [[SLW, BPS], [1, 1]], off=41),
        op=mybir.AluOpType.add)
    tsl = pb.tile([128, BPS], DT.float32, tag="tsl2")
    nc.vector.tensor_scalar_mul(tsl[:], esl[:], NEG)
    nc.vector.tensor_tensor(out=esl[:], in0=esl[:], in1=tsl[:],
                            op=mybir.AluOpType.max)
    wsl = pb.tile([128, BPS], DT.float32, tag="wsl2")
    nc.scalar.activation(wsl[:], esl[:], mybir.ActivationFunctionType.Exp)
    den = pb.tile([128, BPS], DT.float32, tag="den2")
    nc.vector.tensor_tensor(
        out=_brd(den[:], [[1, BPS], [1, 1]]),
        in0=_brd(blk[:], [[nw, BPS], [1, 1]], off=C2),
        in1=_brd(wsl[:], [[1, BPS], [1, 1]]),
        op=mybir.AluOpType.add)
    nc.vector.tensor_scalar_max(den[:], den[:], 1e-30)
    rec = pb.tile([128, BPS], DT.float32, tag="rec2")
    nc.vector.reciprocal(rec[:], den[:])
    num = pb.tile([128, BPS * C2], DT.float32, tag="num2")
    nc.vector.tensor_tensor(
        out=_brd(num[:], [[C2, BPS], [1, C2]]),
        in0=_brd(adsl[:], [[SLW, BPS], [1, C2]]),
        in1=_brd(wsl[:], [[1, BPS], [0, C2]]),
        op=mybir.AluOpType.mult)
    nc.vector.tensor_tensor(
        out=_brd(num[:], [[C2, BPS], [1, C2]]),
        in0=_brd(num[:], [[C2, BPS], [1, C2]]),
        in1=_brd(blk[:], [[nw, BPS], [1, C2]]),
        op=mybir.AluOpType.add)
    o2 = pb.tile([128, BPS * C2], DT.float32, tag="o2")
    nc.vector.tensor_tensor(
        out=_brd(o2[:], [[C2, BPS], [1, C2]]),
        in0=_brd(num[:], [[C2, BPS], [1, C2]]),
        in1=_brd(rec[:], [[1, BPS], [0, C2]]),
        op=mybir.AluOpType.mult)
    nc.vector.tensor_tensor(
        out=_brd(o2[:], [[C2, BPS], [1, C2]]),
        in0=_brd(o2[:], [[C2, BPS], [1, C2]]),
        in1=_brd(b2sb[:], [[0, BPS], [1, C2]]),
        op=mybir.AluOpType.add)
    mx = pb.tile([128, BPS], DT.float32, tag="mx")
    nc.vector.tensor_reduce(
        _brd(mx[:], [[1, BPS], [1, 1]]),
        _brd(o2[:], [[C2, BPS], [1, C2]]),
        axis=mybir.AxisListType.X, op=mybir.AluOpType.max)
    z = pb.tile([128, BPS * C2], DT.float32, tag="z")
    nc.vector.tensor_tensor(
        out=_brd(z[:], [[C2, BPS], [1, C2]]),
        in0=_brd(o2[:], [[C2, BPS], [1, C2]]),
        in1=_brd(mx[:], [[1, BPS], [0, C2]]),
        op=mybir.AluOpType.subtract)
    ez = pb.tile([128, BPS * C2], DT.float32, tag="ez")
    nc.scalar.activation(ez[:], z[:], mybir.ActivationFunctionType.Exp)
    se = pb.tile([128, BPS], DT.float32, tag="se")
    nc.vector.tensor_reduce(
        _brd(se[:], [[1, BPS], [1, 1]]),
        _brd(ez[:], [[C2, BPS], [1, C2]]),
        axis=mybir.AxisListType.X, op=mybir.AluOpType.add)
    lse = pb.tile([128, BPS], DT.float32, tag="lse")
    nc.scalar.activation(lse[:], se[:], mybir.ActivationFunctionType.Ln)
    zo = pb.tile([128, BPS * C2], DT.float32, tag="zo")
    nc.vector.tensor_tensor(
        out=_brd(zo[:], [[C2, BPS], [1, C2]]),
        in0=_brd(z[:], [[C2, BPS], [1, C2]]),
        in1=_brd(lse[:], [[1, BPS], [0, C2]]),
        op=mybir.AluOpType.subtract)
    nc.sync.dma_start(
        bass.AP(out_d.tensor, s * BPS * 128 * C2,
                [[C2, 128], [128 * C2, BPS], [1, C2]]),
        zo[:])


# =================== SPMD runner (bass2jax-based, with timing) ===================

def _run_spmd(nc, in_maps, n_timing_iters=0):
    """Execute the program on NCORES neuron devices via PJRT (axon)."""
    import jax
    from jax.sharding import Mesh, PartitionSpec
    from jax.experimental.shard_map import shard_map
    from concourse import bass2jax
    from concourse.bass2jax import _bass_exec_p, partition_id_tensor
    import time

    bass2jax.install_neuronx_cc_hook()
    assert nc.dbg_addr is None or not nc.dbg_callbacks

    in_names, out_names, out_avals, zero_outs = [], [], [], []
    partition_name = (nc.partition_id_tensor.name
                      if nc.partition_id_tensor else None)
    for alloc in nc.m.functions[0].allocations:
        if not isinstance(alloc, mybir.MemoryLocationSet):
            continue
        name = alloc.memorylocations[0].name
        if alloc.kind == "ExternalInput":
            if name != partition_name:
                in_names.append(name)
        elif alloc.kind == "ExternalOutput":
            out_names.append(name)
            shape = tuple(alloc.tensor_shape)
            dtype = mybir.dt.np(alloc.dtype)
            out_avals.append(jax.core.ShapedArray(shape, dtype))
            zero_outs.append(np.zeros(shape, dtype))
    n_params = len(in_names)
    all_in_names = in_names + out_names + (
        [partition_name] if partition_name else [])

    def _body(*args):
        operands = list(args)
        if partition_name is not None:
            operands.append(partition_id_tensor())
        return tuple(_bass_exec_p.bind(
            *operands,
            out_avals=tuple(out_avals),
            in_names=tuple(all_in_names),
            out_names=tuple(out_names),
            lowering_input_output_aliases=(),
            sim_require_finite=True,
            sim_require_nnan=True,
            nc=nc,
        ))

    devices = jax.devices()[:NCORES]
    mesh = Mesh(np.asarray(devices), ("core",))
    nin = n_params + len(out_names)
    fn = jax.jit(shard_map(_body, mesh=mesh,
                           in_specs=(PartitionSpec("core"),) * nin,
                           out_specs=(PartitionSpec("core"),) * len(out_names),
                           check_rep=False),
                 keep_unused=True)
    sh = jax.sharding.NamedSharding(mesh, PartitionSpec("core"))
    concat_in = [
        jax.device_put(np.concatenate(
            [np.asarray(in_maps[c][name]) for c in range(NCORES)], axis=0), sh)
        for name in in_names
    ]
    concat_zeros = [
        jax.device_put(np.zeros((NCORES * z.shape[0], *z.shape[1:]), z.dtype),
                       sh) for z in zero_outs
    ]
    out_arrs = jax.block_until_ready(fn(*concat_in, *concat_zeros))
    times = []
    for _ in range(n_timing_iters):
        t0 = time.perf_counter()
        r = jax.block_until_ready(fn(*concat_in, *concat_zeros))
        times.append(time.perf_counter() - t0)
        del r
    results = [
        {name: np.asarray(out_arrs[i]).reshape(NCORES, *out_avals[i].shape)[c]
         for i, name in enumerate(out_names)}
        for c in range(NCORES)
    ]
    return results, times


# =================== top-level entry ===================

def kernel(**inputs):
    edge_index = np.asarray(inputs["edge_index"])
    meta, per_core = preprocess(edge_index)
    wts = build_weight_inputs(
        np.asarray(inputs["W1"]), np.asarray(inputs["att_src1"]),
        np.asarray(inputs["att_dst1"]), np.asarray(inputs["bias1"]),
        np.asarray(inputs["W2"]), np.asarray(inputs["att_src2"]),
        np.asarray(inputs["att_dst2"]), np.asarray(inputs["bias2"]))
    x = np.asarray(inputs["x"], _f32)
    in_maps = []
    for c in range(NCORES):
        xs = np.zeros((NP1T * 128, F), BF16)
        xs[:NPC] = x[c * NPC:(c + 1) * NPC].astype(BF16)
        in_maps.append(dict(
            x_sl=xs, W1=wts["W1"], A1=wts["A1"], W2=wts["W2"],
            att2=wts["att2"], b1r=wts["b1r"], b2r=wts["b2r"],
            idxw=per_core[c]["idxw"], dl=per_core[c]["dl"],
            dlT=per_core[c]["dlT"], rowiw=per_core[c]["rowiw"],
            pscatw=per_core[c]["pscatw"]))
    nc = build_program(meta)
    n_iters = int(os.environ.get("GAT_BENCH_ITERS", "0"))
    results, times = _run_spmd(nc, in_maps, n_timing_iters=n_iters)
    global LAST_TIMES
    LAST_TIMES = times
    out = np.zeros((N, C2), _f32)
    for c in range(NCORES):
        pm = per_core[c]["posmap_flat"]
        real = pm >= 0
        out[c * NPC + pm[real]] = results[c]["out"][np.nonzero(real)[0]]
    return out


# revision 33
# speedup vs baseline: 12.2650x; 5.4885x over previous
"""GAT (2-layer, PyG-style) distributed Bass kernel for 8 TRN2 NeuronCores.

Strategy (sharding_hint: 1D node partition by dst), v3:
  - core c owns dst nodes [c*NPC, (c+1)*NPC).
  - dense phase: each core computes table1 rows [h1(64)|a_src1(8)|pad->256B]
    for its node slice; AllGather -> full padded table in every core's DRAM.
    A position-ordered copy [h|asrc|adst] (ad1pos) is written in parallel by
    dma_scatter_add (1024 int16 indices/call; rows unique so add==write on
    the zeroed table).
  - edge phase: self-loop edges are NOT materialized (handled analytically in
    the post-pass, position-aligned).  Remaining edges: host packs dst nodes
    into BLOCKS of <=128 nodes whose edges, split by SOURCE SHARD, fit 2
    single-shard chunks of 128 slots per shard (8-dim FFD; balanced because
    self-loops no longer skew the local shard).  4 blocks = 1 supertile = 64
    chunks, chunk c = shard*8 + block_loc*2 + half.  Per supertile:
      * 8 dma_gather calls (Q7 custom op, 1024 int16 shard-local indices,
        256B rows, spread over 4 SWDGE queues -> ~0.9us/call) pull all 8192
        edge source rows.  vs ~1us per 128-row indirect DMA = ~9x fewer
        Pool-engine descriptor-generation stalls.
      * per-edge a_dst: eadst[e,h] = sposT_chunk^T @ ad_block on TensorE,
        where sposT (pos->edge one-hot) is built by iota/is_equal from a
        K=1 ones-matmul broadcast of the dst-position vector dloc.
      * w = exp(leakyrelu(a_src+a_dst)) (no max subtraction; |logit| < ~3),
        hw = [h*w | w], then per block 16 accumulating matmuls
        blk[128pos, nw] += spos_chunk^T @ hw_chunk (PSUM).
  - block post: fold in the self-loop term (w_sl = exp(lrelu(asrc+adst)),
    num += w_sl*h, den += w_sl) from the position-ordered table, normalize,
    +bias, ELU, W2aug matmul -> layer-2 node rows (dma_scatter_add into the
    zeroed padded table) + position-ordered copy; AllGather #2; same edge
    pass for layer 2; log_softmax; output rows in position order (host
    unpacks by posmap).
All floating-point math runs on-device.  Host work is integer graph
preprocessing (sort/pack/index building) and weight layout rearrangement.
"""
import os
import sys
import numpy as np

try:
    import concourse.bass as bass
except ImportError:  # pragma: no cover
    for p in ("/opt/trn_rl_repo", "/root/.axon_site/_ro/trn_rl_repo"):
        if os.path.isdir(p) and p not in sys.path:
            sys.path.insert(0, p)
    import concourse.bass as bass

import ml_dtypes
import concourse.mybir as mybir
import concourse.tile as tile
import concourse.bacc as bacc
from concourse import library_config
from concourse.masks import make_identity

BF16 = ml_dtypes.bfloat16
DT = mybir.dt

# ---------------- problem config (hardcoded per contract) ----------------
N, E, F = 100000, 1600000, 256
H1, C1 = 8, 8          # layer1 heads x channels (concat -> 64)
C2 = 40                # layer2 single head, 40 classes
NEG = 0.2
NCORES = 8
NPC = N // NCORES      # 12500
NPCT = 12672           # table rows per shard (12544 P1-padded + 128 dump)
ROWP = 128             # padded table row, bf16 elements (= 256B)
TILE_E = 128           # edge slots per tile/chunk
SPB = 2                # chunks per (block, shard)
BPS = 4                # blocks per supertile
K = 64                 # chunks per supertile = 8 shards x 4 blocks x 2
ROW1 = 72              # useful cols layer1: [h1(64) | asrc1(8)]
ROW2 = 41              # useful cols layer2: [h2(40) | asrc2(1)]
NP1T = 99              # P1 tiles (12672 rows / 128)
NP1G = 13              # P1 scatter groups of 8 tiles

_f32 = np.float32


# =================== host-side graph preprocessing ===================

def _pack_blocks(deg2):
    """8-dim FFD: pack nodes into blocks (<=128 nodes, per-shard edge count
    <=SPB*TILE_E).  deg2: [NPC, 8] per-node per-source-shard edge counts."""
    cap = SPB * TILE_E
    order = np.argsort(-deg2.sum(1), kind="stable")
    blocks = []
    rem = np.zeros((0, 8), np.int64)
    npos = np.zeros(0, np.int64)
    open_ids = np.zeros(0, np.int64)
    for n in order:
        dn = deg2[n]
        ok = np.nonzero((rem >= dn).all(1) & (npos < 128))[0]
        if ok.size:
            k = ok[0]
            bi = open_ids[k]
            blocks[bi].append(int(n))
            rem[k] -= dn
            npos[k] += 1
            if npos[k] >= 128 or rem[k].sum() < 3:
                keep = np.arange(rem.shape[0]) != k
                rem, npos, open_ids = rem[keep], npos[keep], open_ids[keep]
        else:
            blocks.append([int(n)])
            rem = np.vstack([rem, (cap - dn)[None]])
            npos = np.append(npos, 1)
            open_ids = np.append(open_ids, len(blocks) - 1)
        if rem.shape[0] > 64:
            drop = np.argmin(rem.sum(1))
            keep = np.arange(rem.shape[0]) != drop
            rem, npos, open_ids = rem[keep], npos[keep], open_ids[keep]
    return blocks


def _wrap16(vals):
    """[n] -> wrapped int16 [128, n/16] layout: index i at [i%16, i//16],
    replicated across the 8 groups of 16 partitions."""
    n = vals.shape[-1]
    v = vals.reshape(*vals.shape[:-1], n // 16, 16)
    v = np.moveaxis(v, -1, -2)  # [..., 16, n//16]
    v = np.concatenate([v] * 8, axis=-2)  # tile to 128 partitions
    return np.ascontiguousarray(v).astype(np.int16)


def preprocess(edge_index):
    # self-loops are handled analytically in the post-pass (position-aligned)
    src = edge_index[0].astype(np.int64)
    dst = edge_index[1].astype(np.int64)

    cores = []
    max_nb = 0
    for c in range(NCORES):
        lo = c * NPC
        m = (dst >= lo) & (dst < lo + NPC)
        s_c, d_c = src[m], dst[m] - lo
        shard = s_c // NPC
        srow = s_c % NPC
        key = d_c * 8 + shard
        order = np.argsort(key, kind="stable")
        srow_s = srow[order]
        deg2 = np.bincount(key, minlength=NPC * 8).reshape(NPC, 8)
        starts = np.zeros(NPC * 8 + 1, np.int64)
        starts[1:] = np.cumsum(deg2.reshape(-1))
        blocks = _pack_blocks(deg2)
        cores.append(dict(srow_s=srow_s, starts=starts, blocks=blocks))
        max_nb = max(max_nb, len(blocks))

    nblocks = -(-max_nb // BPS) * BPS
    assert nblocks * 128 + 128 <= 32767, f"NPOS overflow: {nblocks}"
    S = nblocks // BPS
    NPOS = nblocks * 128
    DUMP = 12544       # dump row base for t2loc (rows [12544, 12672))

    per_core = []
    for cd in cores:
        blocks, starts, srow_s = cd["blocks"], cd["starts"], cd["srow_s"]
        blocks = blocks + [[] for _ in range(nblocks - len(blocks))]
        posmap = np.full((nblocks, 128), -1, np.int64)
        # node/pad -> position; pads & tail -> spread dump positions
        pscat = NPOS + (np.arange(NP1G * 8 * 128, dtype=np.int64) % 128)
        idx_loc = np.zeros((nblocks, 8, SPB * TILE_E), np.int64)
        dpos = np.full((nblocks, 8, SPB * TILE_E), 255, np.int64)
        for b, nodes in enumerate(blocks):
            ptr = np.zeros(8, np.int64)
            for pos, n in enumerate(nodes):
                posmap[b, pos] = n
                pscat[n] = b * 128 + pos
                for sg in range(8):
                    a, z = starts[n * 8 + sg], starts[n * 8 + sg + 1]
                    k = z - a
                    if k:
                        p0 = ptr[sg]
                        idx_loc[b, sg, p0:p0 + k] = srow_s[a:z]
                        dpos[b, sg, p0:p0 + k] = pos
                        ptr[sg] += k
            assert (ptr <= SPB * TILE_E).all()
        # chunk layout: global chunk c = sg*8 + b_loc*SPB + half; call sg
        # covers chunks [sg*8, sg*8+8) = its supertile's 4 blocks x 2 halves
        idx4 = idx_loc.reshape(S, BPS, 8, SPB * TILE_E).transpose(0, 2, 1, 3)
        idxw = _wrap16(idx4.reshape(S, 8, 8 * TILE_E)).reshape(S, 8, 128, 64)
        idxw = np.ascontiguousarray(idxw.transpose(0, 2, 1, 3)).reshape(
            S, 128, 512)
        # dl [S, 128slot, 64chunk]
        dlc = dpos.reshape(S, BPS, 8, SPB, TILE_E).transpose(0, 2, 1, 3, 4)
        dl = np.ascontiguousarray(
            dlc.reshape(S, K, TILE_E).transpose(0, 2, 1)).astype(BF16)
        # dlT [S, 1, 64*128]: [0, c*128 + slot] = dloc(chunk c, slot)
        dlT = np.ascontiguousarray(
            dlc.reshape(S, 1, K * TILE_E)).astype(BF16)
        # rowiw [S, 128, 32]: scatter idx for post blocks: i=b_loc*128+pos
        rowv = posmap.reshape(S, BPS * 128).copy()
        bad = rowv < 0
        rowv[bad] = DUMP + (np.nonzero(bad)[1] % 128)
        rowiw = _wrap16(rowv)
        # pscatw [NP1G, 128, 64]: P1 tile j covers nodes j*128..j*128+127
        pv = pscat[:NP1G * 8 * 128].reshape(NP1G, 8 * 128)
        pscatw = _wrap16(pv)
        per_core.append(dict(idxw=idxw, dl=dl, dlT=dlT, rowiw=rowiw,
                             pscatw=pscatw,
                             posmap_flat=posmap.reshape(-1).copy()))
    meta = dict(S=S, nblocks=nblocks, NPOS=NPOS)
    return meta, per_core


def build_weight_inputs(W1, att_src1, att_dst1, bias1, W2, att_src2, att_dst2,
                        bias2):
    """Pure layout rearrangement of weights (no FP arithmetic)."""
    A1 = np.zeros((64, 16), _f32)
    for h in range(H1):
        A1[h * 8:(h + 1) * 8, h] = att_src1[h]
        A1[h * 8:(h + 1) * 8, 8 + h] = att_dst1[h]
    att2 = np.concatenate([att_src2.T, att_dst2.T], axis=1).astype(_f32)
    b1r = np.broadcast_to(bias1.astype(_f32), (128, 64)).copy()
    b2r = np.broadcast_to(bias2.astype(_f32), (128, C2)).copy()
    return dict(W1=W1.astype(_f32), A1=A1, W2=W2.astype(_f32), att2=att2,
                b1r=b1r, b2r=b2r)


# =================== device program ===================

def _brd(ap, pattern, off=0):
    """Manual AP: keep partition dim, explicit free-dim [step,count] pattern."""
    return bass.AP(ap.tensor, ap.offset + off, [ap.ap[0]] + pattern)


def _ap3(t, chunks, elem, off=0):
    a = t[:]
    return bass.AP(a.tensor, a.offset + off, [a.ap[0], [elem, chunks],
                                              [1, elem]])


def build_program(meta):
    S, NPOS = meta["S"], meta["NPOS"]
    NT = NPCT * NCORES

    nc = bacc.Bacc("TRN2", target_bir_lowering=False, debug=False,
                   enable_asserts=False, num_devices=NCORES,
                   num_swdge_queues=4)

    def din(name, shape, dt):
        return nc.dram_tensor(name, shape, dt, kind="ExternalInput").ap()

    x_sl = din("x_sl", [NP1T * 128, F], DT.bfloat16)
    W1 = din("W1", [F, 64], DT.float32)
    A1 = din("A1", [64, 16], DT.float32)
    W2 = din("W2", [64, C2], DT.float32)
    att2 = din("att2", [C2, 2], DT.float32)
    b1r = din("b1r", [128, 64], DT.float32)
    b2r = din("b2r", [128, C2], DT.float32)
    idxw_d = din("idxw", [S, 128, 512], DT.int16)
    dl_d = din("dl", [S, 128, K], DT.bfloat16)
    dlT_d = din("dlT", [S, 1, K * 128], DT.bfloat16)
    rowiw_d = din("rowiw", [S, 128, 32], DT.int16)
    pscatw_d = din("pscatw", [NP1G, 128, 64], DT.int16)

    out_d = nc.dram_tensor("out", [NPOS, C2], DT.float32,
                           kind="ExternalOutput").ap()

    t1loc = nc.dram_tensor("t1loc", [NPCT, ROWP], DT.bfloat16).ap()
    t1full = nc.dram_tensor("t1full", [NT, ROWP], DT.bfloat16,
                            addr_space="Shared").ap()
    t2loc = nc.dram_tensor("t2loc", [NPCT, ROWP], DT.bfloat16).ap()
    t2full = nc.dram_tensor("t2full", [NT, ROWP], DT.bfloat16,
                            addr_space="Shared").ap()
    ad1pos = nc.dram_tensor("ad1pos", [NPOS + 128, ROWP], DT.bfloat16).ap()
    ad2pos = nc.dram_tensor("ad2pos", [NPOS, 64], DT.bfloat16).ap()

    groups = [list(range(NCORES))]

    with tile.TileContext(nc, num_cores=NCORES) as tc:
        from contextlib import ExitStack
        with ExitStack() as top:
            cpool = top.enter_context(tc.tile_pool(name="const", bufs=1))
            id_f = cpool.tile([128, 128], DT.float32)
            make_identity(nc, id_f[:])
            id_b = cpool.tile([128, 128], DT.bfloat16)
            nc.vector.tensor_copy(id_b[:], id_f[:])
            ioF_i = cpool.tile([128, 128], DT.int16)
            nc.gpsimd.iota(ioF_i[:], pattern=[[1, 128]], base=0,
                           channel_multiplier=0)
            iotaF = cpool.tile([128, 128], DT.bfloat16)
            nc.vector.tensor_copy(iotaF[:], ioF_i[:])
            ioP_i = cpool.tile([128, 1], DT.int16)
            nc.gpsimd.iota(ioP_i[:], pattern=[[0, 1]], base=0,
                           channel_multiplier=1)
            iotaP = cpool.tile([128, 1], DT.bfloat16)
            nc.vector.tensor_copy(iotaP[:], ioP_i[:])
            ones1 = cpool.tile([1, 128], DT.bfloat16)
            nc.vector.memset(ones1[:], 1.0)
            b1sb = cpool.tile([128, 64], DT.float32)
            nc.sync.dma_start(b1sb[:], b1r)
            b2sb = cpool.tile([128, C2], DT.float32)
            nc.sync.dma_start(b2sb[:], b2r)
            # switch Q7 library: enables dma_gather / dma_scatter_add
            nc.gpsimd.load_library(library_config.mlp)

            # ---------- P0: weight prep ----------
            rhs1 = [cpool.tile([128, 80], DT.bfloat16, tag=f"rhs1_{i}",
                               name=f"rhs1_{i}") for i in range(2)]
            rhs2 = cpool.tile([64, 42], DT.bfloat16)
            with tc.tile_pool(name="p0", bufs=1) as p0, \
                 tc.tile_pool(name="p0ps", bufs=1, space="PSUM") as p0ps:
                w1sb = [p0.tile([128, 64], DT.float32, tag=f"w1_{i}",
                                name=f"w1_{i}") for i in range(2)]
                for i in range(2):
                    nc.sync.dma_start(w1sb[i][:], W1[128 * i:128 * (i + 1), :])
                a1sb = p0.tile([64, 16], DT.float32)
                nc.sync.dma_start(a1sb[:], A1)
                w2sb = p0.tile([64, C2], DT.float32)
                nc.sync.dma_start(w2sb[:], W2)
                at2sb = p0.tile([C2, 2], DT.float32)
                nc.sync.dma_start(at2sb[:], att2)
                for i in range(2):
                    tp = p0ps.tile([64, 128], DT.float32, tag="w1t_ps")
                    nc.tensor.transpose(tp[:], w1sb[i][:], id_f[:])
                    w1t = p0.tile([64, 128], DT.float32, tag="w1t")
                    nc.vector.tensor_copy(w1t[:], tp[:])
                    wa = p0ps.tile([128, 16], DT.float32, tag="w1a_ps")
                    nc.tensor.matmul(wa[:], lhsT=w1t[:], rhs=a1sb[:],
                                     start=True, stop=True)
                    nc.vector.tensor_copy(rhs1[i][:, 0:64], w1sb[i][:])
                    nc.vector.tensor_copy(rhs1[i][:, 64:80], wa[:])
                tp2 = p0ps.tile([C2, 64], DT.float32, tag="w2t_ps")
                nc.tensor.transpose(tp2[:], w2sb[:], id_f[:64, :64])
                w2t = p0.tile([C2, 64], DT.float32)
                nc.vector.tensor_copy(w2t[:], tp2[:])
                wa2 = p0ps.tile([64, 2], DT.float32, tag="w2a_ps")
                nc.tensor.matmul(wa2[:], lhsT=w2t[:], rhs=at2sb[:],
                                 start=True, stop=True)
                nc.vector.tensor_copy(rhs2[:, 0:C2], w2sb[:])
                nc.vector.tensor_copy(rhs2[:, C2:C2 + 2], wa2[:])

            # ---------- P0.5: zero scatter-add target tables ----------
            with tc.tile_pool(name="pz", bufs=1) as pz:
                ZW = 4096
                zt = pz.tile([128, ZW], DT.bfloat16)
                nc.vector.memset(zt[:], 0.0)
                for tgt, nelem in ((ad1pos, (NPOS + 128) * ROWP),
                                   (ad2pos, NPOS * 64),
                                   (t2loc, NPCT * ROWP)):
                    done = 0
                    while done < nelem:
                        chunk = min(ZW * 128, nelem - done)
                        w = chunk // 128
                        nc.sync.dma_start(
                            bass.AP(tgt.tensor, done, [[w, 128], [1, w]]),
                            zt[:, 0:w])
                        done += w * 128

            # ---------- P1: dense layer-1 table ----------
            with tc.tile_pool(name="p1", bufs=3) as p1, \
                 tc.tile_pool(name="p1ps", bufs=2, space="PSUM") as p1ps:
                sta8 = opw = None
                for it in range(NP1T):
                    g, gi = divmod(it, 8)
                    if gi == 0:
                        sta8 = p1.tile([128, 8 * ROWP], DT.bfloat16,
                                       tag="sta8", name="sta8")
                        nc.vector.memset(sta8[:], 0.0)
                        opw = p1.tile([128, 64], DT.int16, tag="opw")
                        nc.sync.dma_start(opw[:], pscatw_d[g])
                    xb = p1.tile([128, F], DT.bfloat16, tag="xb")
                    nc.sync.dma_start(xb[:], x_sl[128 * it:128 * (it + 1), :])
                    xT = p1.tile([128, F], DT.bfloat16, tag="xT")
                    ps1 = p1ps.tile([128, 80], DT.float32, tag="ps1")
                    for i in range(2):
                        tp = p1ps.tile([128, 128], DT.bfloat16, tag="xt_ps")
                        nc.tensor.transpose(
                            tp[:], xb[:, 128 * i:128 * (i + 1)], id_b[:])
                        nc.scalar.copy(xT[:, 128 * i:128 * (i + 1)], tp[:])
                    for i in range(2):
                        nc.tensor.matmul(
                            ps1[:], lhsT=xT[:, 128 * i:128 * (i + 1)],
                            rhs=rhs1[i][:], start=(i == 0), stop=(i == 1))
                    st = p1.tile([128, ROWP], DT.bfloat16, tag="st1")
                    nc.scalar.copy(st[:, 0:ROW1], ps1[:, 0:ROW1])
                    nc.scalar.copy(sta8[:, ROWP * gi:ROWP * gi + 80],
                                   ps1[:, 0:80])
                    nc.sync.dma_start(t1loc[128 * it:128 * (it + 1), :], st[:])
                    if gi == 7 or it == NP1T - 1:
                        nc.gpsimd.dma_scatter_add(
                            ad1pos, _ap3(sta8, 8, ROWP), opw[:],
                            1024, 1024, ROWP, queue_num=g % 4)

            # ---------- P2: AllGather table1 ----------
            nc.gpsimd.collective_compute(
                "AllGather", mybir.AluOpType.bypass, replica_groups=groups,
                ins=[t1loc.opt()], outs=[t1full.opt()])

            # ---------- P3: edge pass layer 1 ----------
            edge_pass(nc, tc, meta, 1, idxw_d, dl_d, dlT_d, rowiw_d,
                      t1full, ad1pos, t2loc, ad2pos, None,
                      iotaF, iotaP, ones1, id_b, b1sb, rhs2)

            # ---------- P4: AllGather table2 ----------
            nc.gpsimd.collective_compute(
                "AllGather", mybir.AluOpType.bypass, replica_groups=groups,
                ins=[t2loc.opt()], outs=[t2full.opt()])

            # ---------- P5: edge pass layer 2 ----------
            edge_pass(nc, tc, meta, 2, idxw_d, dl_d, dlT_d, rowiw_d,
                      t2full, ad2pos, None, None, out_d,
                      iotaF, iotaP, ones1, id_b, b2sb, None)

    nc.compile()
    return nc


def edge_pass(nc, tc, meta, layer, idxw_d, dl_d, dlT_d, rowiw_d, tfull,
              adpos, t2loc, ad2pos, out_d, iotaF, iotaP, ones1, id_b,
              bias_sb, rhs2):
    S = meta["S"]
    nh = H1 if layer == 1 else 1          # heads
    nch = 64 if layer == 1 else C2        # message channels
    asrc_c = nch                          # a_src column in table row
    nw = nch + nh                         # hw width: [msgs*w | w]
    from contextlib import ExitStack
    with ExitStack() as ctx:
        pm = ctx.enter_context(tc.tile_pool(name=f"e{layer}m", bufs=3))
        pg = ctx.enter_context(tc.tile_pool(name=f"e{layer}g", bufs=3))
        pw = ctx.enter_context(tc.tile_pool(name=f"e{layer}w", bufs=2))
        pb = ctx.enter_context(tc.tile_pool(name=f"e{layer}b", bufs=2))
        psT = ctx.enter_context(
            tc.tile_pool(name=f"e{layer}pT", bufs=1, space="PSUM"))
        psE = ctx.enter_context(
            tc.tile_pool(name=f"e{layer}pE", bufs=2, space="PSUM"))
        psB = ctx.enter_context(
            tc.tile_pool(name=f"e{layer}pB", bufs=2, space="PSUM"))
        psP = ctx.enter_context(
            tc.tile_pool(name=f"e{layer}pP", bufs=1, space="PSUM"))
        for s in range(S):
            idx = pm.tile([128, 512], DT.int16, tag="idx")
            nc.sync.dma_start(idx[:], idxw_d[s])
            dl = pm.tile([128, K], DT.bfloat16, tag="dl")
            nc.sync.dma_start(dl[:], dl_d[s])
            dlT = pm.tile([1, K * 128], DT.bfloat16, tag="dlT")
            nc.sync.dma_start(dlT[:], dlT_d[s])
            rwi = pm.tile([128, 32], DT.int16, tag="rwi")
            if layer == 1:
                nc.sync.dma_start(rwi[:], rowiw_d[s])
            SLW = 80 if layer == 1 else 42
            RW2 = ROWP if layer == 1 else 64
            ad = pm.tile([128, BPS * nh], DT.bfloat16, tag="ad")
            adsl = pm.tile([128, BPS * SLW], DT.bfloat16, tag="adsl")
            adoff = 72 if layer == 1 else 41
            nc.sync.dma_start(
                ad[:], bass.AP(adpos.tensor, s * BPS * 128 * RW2 + adoff,
                               [[RW2, 128], [128 * RW2, BPS], [1, nh]]))
            nc.sync.dma_start(
                adsl[:], bass.AP(adpos.tensor, s * BPS * 128 * RW2,
                                 [[RW2, 128], [128 * RW2, BPS], [1, SLW]]))

            # gather all 64 chunks: one dma_gather per source shard
            hs = pg.tile([128, K * ROWP], DT.bfloat16, tag="hs")
            for sg in range(8):
                nc.gpsimd.dma_gather(
                    _ap3(hs, 8, ROWP, off=sg * 8 * ROWP),
                    tfull[sg * NPCT:(sg + 1) * NPCT, :],
                    idx[:, sg * 64:(sg + 1) * 64], 1024, 1024, ROWP,
                    queue_num=sg % 4)

            # spos[e, (c,pos)] = (dl[e,c] == pos)
            spos = pw.tile([128, K * 128], DT.bfloat16, tag="spos")
            nc.vector.tensor_tensor(
                out=_brd(spos[:], [[128, K], [1, 128]]),
                in0=_brd(iotaF[:], [[0, K], [1, 128]]),
                in1=_brd(dl[:], [[1, K], [0, 128]]),
                op=mybir.AluOpType.is_equal)
            # sposT[(pos), (c,e)] = (dlT[c,e] == pos), via ones-matmul bcast
            sposT = pw.tile([128, K * 128], DT.bfloat16, tag="sposT")
            for g in range(16):
                pT = psT.tile([128, 512], DT.float32, tag="pT")
                nc.tensor.matmul(pT[:], lhsT=ones1[:],
                                 rhs=dlT[:, g * 512:(g + 1) * 512],
                                 start=True, stop=True)
                nc.vector.tensor_tensor(
                    out=_brd(sposT[:], [[128, 4], [1, 128]], off=g * 512),
                    in0=_brd(iotaP[:], [[0, 4], [0, 128]]),
                    in1=_brd(pT[:], [[128, 4], [1, 128]]),
                    op=mybir.AluOpType.is_equal)

            # eadst via TensorE + e = asrc + eadst; leakyrelu; w = exp(e)
            e = pw.tile([128, K * nh], DT.float32, tag="e")
            for g8 in range(8):
                pE = psE.tile([128, 8 * nh], DT.float32, tag="pE")
                for j in range(8):
                    c = g8 * 8 + j
                    b = (c % 8) // SPB
                    nc.tensor.matmul(
                        pE[:, j * nh:(j + 1) * nh],
                        lhsT=sposT[:, c * 128:(c + 1) * 128],
                        rhs=ad[:, b * nh:(b + 1) * nh],
                        start=True, stop=True, skip_group_check=True)
                nc.vector.tensor_tensor(
                    out=_brd(e[:], [[nh, 8], [1, nh]], off=g8 * 8 * nh),
                    in0=_brd(hs[:], [[ROWP, 8], [1, nh]],
                             off=g8 * 8 * ROWP + asrc_c),
                    in1=_brd(pE[:], [[nh, 8], [1, nh]]),
                    op=mybir.AluOpType.add)
            tmp = pw.tile([128, K * nh], DT.float32, tag="etmp")
            nc.vector.tensor_scalar_mul(tmp[:], e[:], NEG)
            nc.vector.tensor_tensor(out=e[:], in0=e[:], in1=tmp[:],
                                    op=mybir.AluOpType.max)
            w = pw.tile([128, K * nh], DT.bfloat16, tag="w")
            nc.scalar.activation(w[:], e[:], mybir.ActivationFunctionType.Exp)

            # hw = [h*w | w]
            hw = pw.tile([128, K * nw], DT.bfloat16, tag="hw")
            if layer == 1:
                nc.vector.tensor_tensor(
                    out=_brd(hw[:], [[nw, K], [8, 8], [1, 8]]),
                    in0=_brd(hs[:], [[ROWP, K], [8, 8], [1, 8]]),
                    in1=_brd(w[:], [[nh, K], [1, 8], [0, 8]]),
                    op=mybir.AluOpType.mult)
                nc.vector.tensor_copy(
                    _brd(hw[:], [[nw, K], [1, 8]], off=64), w[:])
            else:
                nc.vector.tensor_tensor(
                    out=_brd(hw[:], [[nw, K], [1, C2]]),
                    in0=_brd(hs[:], [[ROWP, K], [1, C2]]),
                    in1=_brd(w[:], [[1, K], [0, C2]]),
                    op=mybir.AluOpType.mult)
                nc.vector.tensor_copy(
                    _brd(hw[:], [[nw, K], [1, 1]], off=C2), w[:])

            # per block: 16 accumulating matmuls into one wide PSUM tile
            blk = psB.tile([128, BPS * nw], DT.float32, tag="blk")
            for b in range(BPS):
                for q in range(16):
                    c = (q // 2) * 8 + b * SPB + (q % 2)
                    nc.tensor.matmul(
                        blk[:, b * nw:(b + 1) * nw],
                        lhsT=spos[:, c * 128:(c + 1) * 128],
                        rhs=hw[:, c * nw:(c + 1) * nw],
                        start=(q == 0), stop=(q == 15),
                        skip_group_check=True)
            if layer == 1:
                _post1(nc, s, blk, adsl, pb, psP, rwi, t2loc, ad2pos,
                       id_b, bias_sb, rhs2)
            else:
                _post2(nc, s, blk, adsl, pb, out_d, bias_sb)


def _post1(nc, s, blk, adsl, pb, psP, rwi, t2loc, ad2pos, id_b, b1sb,
           rhs2):
    """Finalize all 4 blocks of a supertile (layer 1), emit table-2 rows.

    blk:  [128, 4*72] PSUM, per block [msgs(64)|denoms(8)].
    adsl: [128, 4*80] position-aligned [h|asrc|adst] rows (self-loop fold).
    """
    SLW = 80
    nw = 72
    st2w = pb.tile([128, BPS * ROWP], DT.bfloat16, tag="st2w")
    nc.vector.memset(st2w[:], 0.0)
    staw = pb.tile([128, BPS * 64], DT.bfloat16, tag="staw")
    nc.vector.memset(staw[:], 0.0)
    esl = pb.tile([128, BPS * 8], DT.float32, tag="esl")
    nc.vector.tensor_tensor(
        out=_brd(esl[:], [[8, BPS], [1, 8]]),
        in0=_brd(adsl[:], [[SLW, BPS], [1, 8]], off=64),
        in1=_brd(adsl[:], [[SLW, BPS], [1, 8]], off=72),
        op=mybir.AluOpType.add)
    tsl = pb.tile([128, BPS * 8], DT.float32, tag="tsl")
    nc.vector.tensor_scalar_mul(tsl[:], esl[:], NEG)
    nc.vector.tensor_tensor(out=esl[:], in0=esl[:], in1=tsl[:],
                            op=mybir.AluOpType.max)
    wsl = pb.tile([128, BPS * 8], DT.float32, tag="wsl")
    nc.scalar.activation(wsl[:], esl[:], mybir.ActivationFunctionType.Exp)
    den = pb.tile([128, BPS * 8], DT.float32, tag="den")
    nc.vector.tensor_tensor(
        out=_brd(den[:], [[8, BPS], [1, 8]]),
        in0=_brd(blk[:], [[nw, BPS], [1, 8]], off=64),
        in1=_brd(wsl[:], [[8, BPS], [1, 8]]),
        op=mybir.AluOpType.add)
    nc.vector.tensor_scalar_max(den[:], den[:], 1e-30)
    rec = pb.tile([128, BPS * 8], DT.float32, tag="rec")
    nc.vector.reciprocal(rec[:], den[:])
    num = pb.tile([128, BPS * 64], DT.float32, tag="num")
    nc.vector.tensor_tensor(
        out=_brd(num[:], [[64, BPS], [8, 8], [1, 8]]),
        in0=_brd(adsl[:], [[SLW, BPS], [8, 8], [1, 8]]),
        in1=_brd(wsl[:], [[8, BPS], [1, 8], [0, 8]]),
        op=mybir.AluOpType.mult)
    nc.vector.tensor_tensor(
        out=_brd(num[:], [[64, BPS], [1, 64]]),
        in0=_brd(num[:], [[64, BPS], [1, 64]]),
        in1=_brd(blk[:], [[nw, BPS], [1, 64]]),
        op=mybir.AluOpType.add)
    hin = pb.tile([128, BPS * 64], DT.float32, tag="hin")
    nc.vector.tensor_tensor(
        out=_brd(hin[:], [[64, BPS], [8, 8], [1, 8]]),
        in0=_brd(num[:], [[64, BPS], [8, 8], [1, 8]]),
        in1=_brd(rec[:], [[8, BPS], [1, 8], [0, 8]]),
        op=mybir.AluOpType.mult)
    nc.vector.tensor_tensor(
        out=_brd(hin[:], [[64, BPS], [1, 64]]),
        in0=_brd(hin[:], [[64, BPS], [1, 64]]),
        in1=_brd(b1sb[:], [[0, BPS], [1, 64]]),
        op=mybir.AluOpType.add)
    # ELU = max(x,0) + exp(min(x,0)) - 1
    emn = pb.tile([128, BPS * 64], DT.float32, tag="emn")
    nc.vector.tensor_scalar_min(emn[:], hin[:], 0.0)
    nc.scalar.activation(emn[:], emn[:], mybir.ActivationFunctionType.Exp)
    nc.vector.tensor_scalar_max(hin[:], hin[:], 0.0)
    nc.vector.tensor_tensor(out=hin[:], in0=hin[:], in1=emn[:],
                            op=mybir.AluOpType.add)
    helu = pb.tile([128, BPS * 64], DT.bfloat16, tag="helu")
    nc.vector.tensor_scalar_add(helu[:], hin[:], -1.0)
    for b in range(BPS):
        htp = psP.tile([64, 128], DT.bfloat16, tag="htp")
        nc.tensor.transpose(htp[:], helu[:, b * 64:(b + 1) * 64], id_b[:])
        hts = pb.tile([64, 128], DT.bfloat16, tag="hts")
        nc.scalar.copy(hts[:], htp[:])
        h2ps = psP.tile([128, 42], DT.float32, tag="h2ps")
        nc.tensor.matmul(h2ps[:], lhsT=hts[:], rhs=rhs2[:], start=True,
                         stop=True)
        nc.scalar.copy(st2w[:, b * ROWP:b * ROWP + ROW2], h2ps[:, 0:ROW2])
        nc.scalar.copy(staw[:, b * 64:b * 64 + 42], h2ps[:, 0:42])
    nc.gpsimd.dma_scatter_add(
        t2loc, _ap3(st2w, BPS, ROWP), rwi[:], 512, 512, ROWP,
        queue_num=s % 4)
    nc.sync.dma_start(
        bass.AP(ad2pos.tensor, s * BPS * 128 * 64,
                [[64, 128], [128 * 64, BPS], [1, 64]]),
        staw[:])


def _post2(nc, s, blk, adsl, pb, out_d, b2sb):
    """Finalize all 4 blocks of a supertile (layer 2): log_softmax rows.

    blk:  [128, 4*41] PSUM, per block [msgs(40)|denom(1)].
    adsl: [128, 4*42] position-aligned [h2|asrc2|adst2] rows.
    """
    SLW = 42
    nw = C2 + 1
    esl = pb.tile([128, BPS], DT.float32, tag="esl2")
    nc.vector.tensor_tensor(
        out=_brd(esl[:], [[1, BPS], [1, 1]]),
        in0=_brd(adsl[:], [[SLW, BPS], [1, 1]], off=40),
        in1=_brd(adsl[:], [[SLW, BPS], [1, 1]], off=41),
        op=mybir.AluOpType.add)
    tsl = pb.tile([128, BPS], DT.float32, tag="tsl2")
    nc.vector.tensor_scalar_mul(tsl[:], esl[:], NEG)
    nc.vector.tensor_tensor(out=esl[:], in0=esl[:], in1=tsl[:],
                            op=mybir.AluOpType.max)
    wsl = pb.tile([128, BPS], DT.float32, tag="wsl2")
    nc.scalar.activation(wsl[:], esl[:], mybir.ActivationFunctionType.Exp)
    den = pb.tile([128, BPS], DT.float32, tag="den2")
    nc.vector.tensor_tensor(
        out=_brd(den[:], [[1, BPS], [1, 1]]),
        in0=_brd(blk[:], [[nw, BPS], [1, 1]], off=C2),
        in1=_brd(wsl[:], [[1, BPS], [1, 1]]),
        op=mybir.AluOpType.add)
    nc.vector.tensor_scalar_max(den[:], den[:], 1e-30)
    rec = pb.tile([128, BPS], DT.float32, tag="rec2")
    nc.vector.reciprocal(rec[:], den[:])
    num = pb.tile([128, BPS * C2], DT.float32, tag="num2")
    nc.vector.tensor_tensor(
        out=_brd(num[:], [[C2, BPS], [1, C2]]),
        in0=_brd(adsl[:], [[SLW, BPS], [1, C2]]),
        in1=_brd(wsl[:], [[1, BPS], [0, C2]]),
        op=mybir.AluOpType.mult)
    nc.vector.tensor_tensor(
        out=_brd(num[:], [[C2, BPS], [1, C2]]),
        in0=_brd(num[:], [[C2, BPS], [1, C2]]),
        in1=_brd(blk[:], [[nw, BPS], [1, C2]]),
        op=mybir.AluOpType.add)
    o2 = pb.tile([128, BPS * C2], DT.float32, tag="o2")
    nc.vector.tensor_tensor(
        out=_brd(o2[:], [[C2, BPS], [1, C2]]),
        in0=_brd(num[:], [[C2, BPS], [1, C2]]),
        in1=_brd(rec[:], [[1, BPS], [0, C2]]),
        op=mybir.AluOpType.mult)
    nc.vector.tensor_tensor(
        out=_brd(o2[:], [[C2, BPS], [1, C2]]),
        in0=_brd(o2[:], [[C2, BPS], [1, C2]]),
        in1=_brd(b2sb[:], [[0, BPS], [1, C2]]),
        op=mybir.AluOpType.add)
    mx = pb.tile([128, BPS], DT.float32, tag="mx")
    nc.vector.tensor_reduce(
        _brd(mx[:], [[1, BPS], [1, 1]]),
        _brd(o2[:], [[C2, BPS], [1, C2]]),
        axis=mybir.AxisListType.X, op=mybir.AluOpType.max)
    z = pb.tile([128, BPS * C2], DT.float32, tag="z")
    nc.vector.tensor_tensor(
        out=_brd(z[:], [[C2, BPS], [1, C2]]),
        in0=_brd(o2[:], [[C2, BPS], [1, C2]]),
        in1=_brd(mx[:], [[1, BPS], [0, C2]]),
        op=mybir.AluOpType.subtract)
    ez = pb.tile([128, BPS * C2], DT.float32, tag="ez")
    nc.scalar.activation(ez[:], z[:], mybir.ActivationFunctionType.Exp)
    se = pb.tile([128, BPS], DT.float32, tag="se")
    nc.vector.tensor_reduce(
        _brd(se[:], [[1, BPS], [1, 1]]),
        _brd(ez[:], [[C2, BPS], [1, C2]]),
        axis=mybir.AxisListType.X, op=mybir.AluOpType.add)
    lse = pb.tile([128, BPS], DT.float32, tag="lse")
    nc.scalar.activation(lse[:], se[:], mybir.ActivationFunctionType.Ln)
    zo = pb.tile([128, BPS * C2], DT.float32, tag="zo")
    nc.vector.tensor_tensor(
        out=_brd(zo[:], [[C2, BPS], [1, C2]]),
        in0=_brd(z[:], [[C2, BPS], [1, C2]]),
        in1=_brd(lse[:], [[1, BPS], [0, C2]]),
        op=mybir.AluOpType.subtract)
    nc.sync.dma_start(
        bass.AP(out_d.tensor, s * BPS * 128 * C2,
                [[C2, 128], [128 * C2, BPS], [1, C2]]),
        zo[:])


# =================== SPMD runner (bass2jax-based, with timing) ===================

def _run_spmd(nc, in_maps, n_timing_iters=0):
    """Execute the program on NCORES neuron devices via PJRT (axon)."""
    import jax
    from jax.sharding import Mesh, PartitionSpec
    from jax.experimental.shard_map import shard_map
    from concourse import bass2jax
    from concourse.bass2jax import _bass_exec_p, partition_id_tensor
    import time

    bass2jax.install_neuronx_cc_hook()
    assert nc.dbg_addr is None or not nc.dbg_callbacks

    in_names, out_names, out_avals, zero_outs = [], [], [], []
    partition_name = (nc.partition_id_tensor.name
                      if nc.partition_id_tensor else None)
    for alloc in nc.m.functions[0].allocations:
        if not isinstance(alloc, mybir.MemoryLocationSet):
            continue
        name = alloc.memorylocations[0].name
        if alloc.kind == "ExternalInput":
            if name != partition_name:
                in_names.append(name)
        elif alloc.kind == "ExternalOutput":
            out_names.append(name)
            shape = tuple(alloc.tensor_shape)
            dtype = mybir.dt.np(alloc.dtype)
            out_avals.append(jax.core.ShapedArray(shape, dtype))
            zero_outs.append(np.zeros(shape, dtype))
    n_params = len(in_names)
    all_in_names = in_names + out_names + (
        [partition_name] if partition_name else [])

    def _body(*args):
        operands = list(args)
        if partition_name is not None:
            operands.append(partition_id_tensor())
        return tuple(_bass_exec_p.bind(
            *operands,
            out_avals=tuple(out_avals),
            in_names=tuple(all_in_names),
            out_names=tuple(out_names),
            lowering_input_output_aliases=(),
            sim_require_finite=True,
            sim_require_nnan=True,
            nc=nc,
        ))

    devices = jax.devices()[:NCORES]
    mesh = Mesh(np.asarray(devices), ("core",))
    nin = n_params + len(out_names)
    fn = jax.jit(shard_map(_body, mesh=mesh,
                           in_specs=(PartitionSpec("core"),) * nin,
                           out_specs=(PartitionSpec("core"),) * len(out_names),
                           check_rep=False),
                 keep_unused=True)
    sh = jax.sharding.NamedSharding(mesh, PartitionSpec("core"))
    concat_in = [
        jax.device_put(np.concatenate(
            [np.asarray(in_maps[c][name]) for c in range(NCORES)], axis=0), sh)
        for name in in_names
    ]
    concat_zeros = [
        jax.device_put(np.zeros((NCORES * z.shape[0], *z.shape[1:]), z.dtype),
                       sh) for z in zero_outs
    ]
    out_arrs = jax.block_until_ready(fn(*concat_in, *concat_zeros))
    times = []
    for _ in range(n_timing_iters):
        t0 = time.perf_counter()
        r = jax.block_until_ready(fn(*concat_in, *concat_zeros))
        times.append(time.perf_counter() - t0)
        del r
    results = [
        {name: np.asarray(out_arrs[i]).reshape(NCORES, *out_avals[i].shape)[c]
         for i, name in enumerate(out_names)}
        for c in range(NCORES)
    ]
    return results, times


# =================== top-level entry ===================

def kernel(**inputs):
    edge_index = np.asarray(inputs["edge_index"])
    meta, per_core = preprocess(edge_index)
    wts = build_weight_inputs(
        np.asarray(inputs["W1"]), np.asarray(inputs["att_src1"]),
        np.asarray(inputs["att_dst1"]), np.asarray(inputs["bias1"]),
        np.asarray(inputs["W2"]), np.asarray(inputs["att_src2"]),
        np.asarray(inputs["att_dst2"]), np.asarray(inputs["bias2"]))
    x = np.asarray(inputs["x"], _f32)
    in_maps = []
    for c in range(NCORES):
        xs = np.zeros((NP1T * 128, F), BF16)
        xs[:NPC] = x[c * NPC:(c + 1) * NPC].astype(BF16)
        in_maps.append(dict(
            x_sl=xs, W1=wts["W1"], A1=wts["A1"], W2=wts["W2"],
            att2=wts["att2"], b1r=wts["b1r"], b2r=wts["b2r"],
            idxw=per_core[c]["idxw"], dl=per_core[c]["dl"],
            dlT=per_core[c]["dlT"], rowiw=per_core[c]["rowiw"],
            pscatw=per_core[c]["pscatw"]))
    nc = build_program(meta)
    n_iters = int(os.environ.get("GAT_BENCH_ITERS", "0"))
    results, times = _run_spmd(nc, in_maps, n_timing_iters=n_iters)
    global LAST_TIMES
    LAST_TIMES = times
    out = np.zeros((N, C2), _f32)
    for c in range(NCORES):
        pm = per_core[c]["posmap_flat"]
        real = pm >= 0
        out[c * NPC + pm[real]] = results[c]["out"][np.nonzero(real)[0]]
    return out
